# revision 1
# baseline (speedup 1.0000x reference)
"""Trainium2 Bass kernel for nn_ColorLoss: mean CIEDE2000 over RGB images.

Sharding: pure data parallel over batch — 16 images, 8 cores, 2 images/core.
Each core computes per-partition partial sums of deltaE; host reduces.

Math restructuring vs the jax reference (validated to ~2e-6 rel in proto.py):
- sRGB gamma + Lab f() branches via continuous-junction min/max tricks.
- pow/cbrt/sqrt via exp(k*ln(x)) (natural_log_exp ACT set); arctan/sin via
  the trig_and_small ACT set => only 2 activation table sets in play.
- dH = sign(b2*a1p - a2p*b1) * sqrt(2*(C1p*C2p - a1p*a2p - b1*b2))
  (half-angle identity, no per-image hue angles, wrap-free).
- hbar = atan2_[0,360)(b1*C2p + b2*C1p, a1p*C2p + a2p*C1p) (bisector).
- T cosines: mod-360 range reduction via the fp32 round-to-nearest magic
  constant, then Sin (HW Sin is only valid on [-pi, pi]).
- Reciprocals via the custom-DVE reciprocal_approx_fast (~3e-6 rel).

SBUF slots are hand-allocated (tag reuse after last read) so a whole
(128,1024) chunk pipeline fits: wk 27 tags * 4KB + wk2 7 tags * 2 * 4KB
+ io 6 * 4KB = 188KB; vm/targ scratch live in PSUM.
"""
import sys

sys.path.insert(0, '/opt/trn_rl_repo')

import math

import numpy as np

import concourse.bacc as bacc
import concourse.mybir as mybir
import concourse.tile as tile

AF = mybir.ActivationFunctionType
OP = mybir.AluOpType
F32 = mybir.dt.float32

B, C, H, W = 16, 3, 512, 512
NCORE = 8
IPC = B // NCORE            # images per core
PLANE = H * W               # elements per channel plane
PF = PLANE // 128           # free elems per partition for a full plane (2048)
FCH = 1024                  # free-dim chunk size
NCH_IMG = PF // FCH         # chunks per image
NCHUNK = IPC * NCH_IMG      # accumulator columns per core

# constants
M = [[0.412453, 0.357580, 0.180423],
     [0.212671, 0.715160, 0.072169],
     [0.019334, 0.119193, 0.950227]]
WHITE = [0.95047, 1.0, 1.08883]
EPS = 0.008856
C0G = 0.04045
L0 = C0G / 12.92
K_F = 16.0 / 116.0 - EPS ** (1.0 / 3.0)
KP7 = 25.0 ** 7
B7 = 7.0 * math.log(0.5)
B35 = 3.5 * math.log(0.5)
MAGIC = float(np.float32(1.5 * 2 ** 23))
DEG = 180.0 / math.pi
TINY = 1e-30
# deg->rad that cannot exceed pi in f32 after *180 (CoreSim range assert)
D2R = math.pi / 180.0 * (1.0 - 3e-7)

_NC_CACHE = {}


def _emit_lab(nc, wk, wk2, planes, slots):
    """RGB (3 plane APs in SBUF) -> (L, a, b) tiles in the given wk slots."""
    P, F = 128, FCH
    sL, sA, sB = slots
    lins = []
    for ci, cp in enumerate(planes):
        u = wk2.tile([P, F], F32, tag="gu")
        # u = max(c, c0) + 0.055
        nc.vector.tensor_scalar(out=u[:], in0=cp[:], scalar1=C0G,
                                scalar2=0.055, op0=OP.max, op1=OP.add)
        # p = ((max(c,c0)+0.055)/1.055)^2.4 = exp(2.4*ln(u/1.055))
        nc.scalar.activation(u[:], u[:], AF.Ln, scale=1.0 / 1.055)
        nc.scalar.activation(u[:], u[:], AF.Exp, scale=2.4)
        m = wk2.tile([P, F], F32, tag="gm")
        # m = min(c, c0) / 12.92
        nc.vector.tensor_scalar(out=m[:], in0=cp[:], scalar1=C0G,
                                scalar2=1.0 / 12.92, op0=OP.min, op1=OP.mult)
        lin = wk.tile([P, F], F32, tag=f"lin{ci}")
        # lin = (m - L0) + p
        nc.vector.scalar_tensor_tensor(out=lin[:], in0=m[:], scalar=-L0,
                                       in1=u[:], op0=OP.add, op1=OP.add)
        lins.append(lin)
    lr, lg, lb = lins
    fs = []
    for k in range(3):
        m0, m1, m2 = M[k]
        S = m0 / WHITE[k]
        t2 = wk2.tile([P, F], F32, tag="t2")
        # t2 = r + g*m1/m0 + b*m2/m0;  t = S*t2 is the normalized XYZ coord
        nc.vector.scalar_tensor_tensor(out=t2[:], in0=lg[:], scalar=m1 / m0,
                                       in1=lr[:], op0=OP.mult, op1=OP.add)
        nc.vector.scalar_tensor_tensor(out=t2[:], in0=lb[:], scalar=m2 / m0,
                                       in1=t2[:], op0=OP.mult, op1=OP.add)
        fv = wk2.tile([P, F], F32, tag="fv")
        # v = max(t2, eps/S); cb = cbrt(S*v) = exp(ln(S*v)/3)
        nc.gpsimd.tensor_scalar(out=fv[:], in0=t2[:], scalar1=EPS / S,
                                scalar2=None, op0=OP.max)
        nc.scalar.activation(fv[:], fv[:], AF.Ln, scale=S)
        nc.scalar.activation(fv[:], fv[:], AF.Exp, scale=1.0 / 3.0)
        fm = wk2.tile([P, F], F32, tag="fm")
        # fm = min(t2, eps/S) * 7.787*S
        nc.vector.tensor_scalar(out=fm[:], in0=t2[:], scalar1=EPS / S,
                                scalar2=7.787 * S, op0=OP.min, op1=OP.mult)
        f = wk.tile([P, F], F32, tag=f"f{k}")
        # f = (fm + K_F) + cb
        nc.vector.scalar_tensor_tensor(out=f[:], in0=fm[:], scalar=K_F,
                                       in1=fv[:], op0=OP.add, op1=OP.add)
        fs.append(f)
    fx, fy, fz = fs
    Lt = wk.tile([P, F], F32, tag=sL)
    nc.vector.tensor_scalar(out=Lt[:], in0=fy[:], scalar1=116.0,
                            scalar2=-16.0, op0=OP.mult, op1=OP.add)
    at = wk.tile([P, F], F32, tag=sA)
    nc.gpsimd.tensor_tensor(out=at[:], in0=fx[:], in1=fy[:], op=OP.subtract)
    nc.gpsimd.tensor_scalar(out=at[:], in0=at[:], scalar1=500.0,
                            scalar2=None, op0=OP.mult)
    bt = wk.tile([P, F], F32, tag=sB)
    nc.gpsimd.tensor_tensor(out=bt[:], in0=fy[:], in1=fz[:], op=OP.subtract)
    nc.gpsimd.tensor_scalar(out=bt[:], in0=bt[:], scalar1=200.0,
                            scalar2=None, op0=OP.mult)
    return Lt, at, bt


def _emit_sqrt(nc, t, scale=1.0):
    """t <- sqrt(scale*t) in place via exp(0.5*ln(scale*t + tiny))."""
    nc.scalar.activation(t[:], t[:], AF.Ln, scale=scale, bias=TINY)
    nc.scalar.activation(t[:], t[:], AF.Exp, scale=0.5)


def _emit_chunk(nc, iop, wk, wk2, psp, t_out, t_lab, img, ci, acc, chunk):
    P, F = 128, FCH
    sl = slice(ci * FCH, (ci + 1) * FCH)

    # ---- load 6 channel-plane chunks --------------------------------------
    def load(t_dram, ch, tag):
        view = t_dram[img, ch].rearrange("(p n) w -> p (n w)", p=128)
        tl = iop.tile([P, F], F32, tag=tag)
        nc.sync.dma_start(tl[:], view[:, sl])
        return tl

    lab_planes = [load(t_lab, ch, f"in_l{ch}") for ch in range(3)]
    out_planes = [load(t_out, ch, f"in_o{ch}") for ch in range(3)]

    # ---- RGB -> Lab for both images (lab1 = labels, lab2 = outputs) -------
    L1, a1, b1 = _emit_lab(nc, wk, wk2, lab_planes, ("sL1", "sA1", "sB1"))
    L2, a2, b2 = _emit_lab(nc, wk, wk2, out_planes, ("sL2", "sA2", "sB2"))

    V, G, S = nc.vector, nc.gpsimd, nc.scalar

    # ---- SL chain (early: frees L slots) ----------------------------------
    lsum = wk.tile([P, F], F32, tag="sSL")
    G.tensor_tensor(out=lsum[:], in0=L1[:], in1=L2[:], op=OP.add)
    dL = wk.tile([P, F], F32, tag="sDL")
    G.tensor_tensor(out=dL[:], in0=L2[:], in1=L1[:], op=OP.subtract)
    # q = (0.5*lsum - 50)^2 = (Lbar-50)^2
    S.activation(lsum[:], lsum[:], AF.Square, scale=0.5, bias=-50.0)
    lnq = wk.tile([P, F], F32, tag="sLQ")
    S.activation(lnq[:], lsum[:], AF.Ln, bias=TINY)
    S.activation(lsum[:], lsum[:], AF.Ln, bias=20.0)       # ln(q+20)
    # esl = exp(ln(q) - 0.5*ln(q+20)) = q/sqrt(20+q)
    V.scalar_tensor_tensor(out=lsum[:], in0=lsum[:], scalar=-0.5,
                           in1=lnq[:], op0=OP.mult, op1=OP.add)
    S.activation(lsum[:], lsum[:], AF.Exp)
    V.tensor_scalar(out=lsum[:], in0=lsum[:], scalar1=0.015,
                    scalar2=1.0, op0=OP.mult, op1=OP.add)  # SL
    V.reciprocal_approx_fast(out=lsum[:], in_=lsum[:])     # 1/SL
    G.tensor_tensor(out=dL[:], in0=dL[:], in1=lsum[:], op=OP.mult)  # tL
    S.activation(dL[:], dL[:], AF.Square)                  # tL^2

    # ---- C1, C2, G, a1p/a2p, C1p/C2p --------------------------------------
    b1sq = wk.tile([P, F], F32, tag="sBS1")
    S.activation(b1sq[:], b1[:], AF.Square)
    b2sq = wk.tile([P, F], F32, tag="sBS2")
    S.activation(b2sq[:], b2[:], AF.Square)
    c1 = wk.tile([P, F], F32, tag="sC1")
    S.activation(c1[:], a1[:], AF.Square)
    V.tensor_tensor(out=c1[:], in0=c1[:], in1=b1sq[:], op=OP.add)
    _emit_sqrt(nc, c1)                                     # C1
    c2 = wk.tile([P, F], F32, tag="sC2")
    S.activation(c2[:], a2[:], AF.Square)
    V.tensor_tensor(out=c2[:], in0=c2[:], in1=b2sq[:], op=OP.add)
    _emit_sqrt(nc, c2)                                     # C2

    tsum = wk.tile([P, F], F32, tag="sTS")
    G.tensor_tensor(out=tsum[:], in0=c1[:], in1=c2[:], op=OP.add)
    S.activation(tsum[:], tsum[:], AF.Ln, bias=TINY)       # ln(C1+C2)
    c7 = wk.tile([P, F], F32, tag="sC7")
    S.activation(c7[:], tsum[:], AF.Exp, scale=7.0, bias=B7)   # Cbar^7
    S.activation(c7[:], c7[:], AF.Ln, bias=KP7)            # ln(c7+25^7)
    # sr = exp(0.5*(7*lnt - lnd) + B35) = sqrt(Cbar^7/(Cbar^7+25^7))
    V.scalar_tensor_tensor(out=c7[:], in0=tsum[:], scalar=7.0,
                           in1=c7[:], op0=OP.mult, op1=OP.subtract)
    S.activation(c7[:], c7[:], AF.Exp, scale=0.5, bias=B35)
    V.tensor_scalar(out=c7[:], in0=c7[:], scalar1=-0.5,
                    scalar2=1.5, op0=OP.mult, op1=OP.add)  # 1+G
    V.tensor_tensor(out=a1[:], in0=a1[:], in1=c7[:], op=OP.mult)  # a1p
    V.tensor_tensor(out=a2[:], in0=a2[:], in1=c7[:], op=OP.mult)  # a2p
    a1p, a2p = a1, a2

    c1p = wk.tile([P, F], F32, tag="sC1P")
    S.activation(c1p[:], a1p[:], AF.Square)
    V.tensor_tensor(out=c1p[:], in0=c1p[:], in1=b1sq[:], op=OP.add)
    _emit_sqrt(nc, c1p)                                    # C1p
    c2p = wk.tile([P, F], F32, tag="sC2P")
    S.activation(c2p[:], a2p[:], AF.Square)
    V.tensor_tensor(out=c2p[:], in0=c2p[:], in1=b2sq[:], op=OP.add)
    _emit_sqrt(nc, c2p)                                    # C2p

    prodC = wk.tile([P, F], F32, tag="sPC")
    G.tensor_tensor(out=prodC[:], in0=c1p[:], in1=c2p[:], op=OP.mult)
    mz = wk.tile([P, F], F32, tag="sMZ")
    G.tensor_scalar(out=mz[:], in0=prodC[:], scalar1=0.0, scalar2=None,
                    op0=OP.is_gt)

    # ---- dH magnitude (slot sC1) and sign (slot sC2) ----------------------
    dot = wk.tile([P, F], F32, tag="sC1")
    G.tensor_tensor(out=dot[:], in0=a1p[:], in1=a2p[:], op=OP.mult)
    sc2 = wk2.tile([P, F], F32, tag="sc2")
    G.tensor_tensor(out=sc2[:], in0=b1[:], in1=b2[:], op=OP.mult)
    G.tensor_tensor(out=dot[:], in0=dot[:], in1=sc2[:], op=OP.add)
    G.tensor_tensor(out=dot[:], in0=prodC[:], in1=dot[:], op=OP.subtract)
    G.tensor_scalar(out=dot[:], in0=dot[:], scalar1=0.0, scalar2=None,
                    op0=OP.max)
    _emit_sqrt(nc, dot, scale=2.0)                         # |dH|
    rootH = dot

    sd = wk.tile([P, F], F32, tag="sC2")
    G.tensor_tensor(out=sd[:], in0=b2[:], in1=a1p[:], op=OP.mult)
    sc2b = wk2.tile([P, F], F32, tag="sc2")
    G.tensor_tensor(out=sc2b[:], in0=a2p[:], in1=b1[:], op=OP.mult)
    G.tensor_tensor(out=sd[:], in0=sd[:], in1=sc2b[:], op=OP.subtract)
    S.activation(sd[:], sd[:], AF.Sign)                    # sign(sin dh)
    sg = sd

    # ---- bisector vector for hbar: ny (slot sTS), nx (slot sC7) -----------
    ny = wk.tile([P, F], F32, tag="sTS")
    G.tensor_tensor(out=ny[:], in0=b1[:], in1=c2p[:], op=OP.mult)
    sc2c = wk2.tile([P, F], F32, tag="sc2")
    G.tensor_tensor(out=sc2c[:], in0=b2[:], in1=c1p[:], op=OP.mult)
    G.tensor_tensor(out=ny[:], in0=ny[:], in1=sc2c[:], op=OP.add)
    nx = wk.tile([P, F], F32, tag="sC7")
    G.tensor_tensor(out=nx[:], in0=a1p[:], in1=c2p[:], op=OP.mult)
    sc2d = wk2.tile([P, F], F32, tag="sc2")
    G.tensor_tensor(out=sc2d[:], in0=a2p[:], in1=c1p[:], op=OP.mult)
    G.tensor_tensor(out=nx[:], in0=nx[:], in1=sc2d[:], op=OP.add)
    # guard prodC==0: nx += (1-mz) so atan2 sees (0,1) -> hbar=0
    V.affine_then_add(out=nx[:], in0=mz[:], in1=nx[:], scale=-1.0, bias=1.0)

    dC = wk.tile([P, F], F32, tag="sDC")
    G.tensor_tensor(out=dC[:], in0=c2p[:], in1=c1p[:], op=OP.subtract)
    ts2t = wk.tile([P, F], F32, tag="sT2")
    G.tensor_tensor(out=ts2t[:], in0=c1p[:], in1=c2p[:], op=OP.add)

    # ---- hbar = atan2_[0,360)(ny, nx) -------------------------------------
    aa = wk.tile([P, F], F32, tag="sL1")
    S.activation(aa[:], nx[:], AF.Abs)
    ab = wk.tile([P, F], F32, tag="sL2")
    S.activation(ab[:], ny[:], AF.Abs)
    ms = wk.tile([P, F], F32, tag="sMZ2")
    V.tensor_tensor(out=ms[:], in0=ab[:], in1=aa[:], op=OP.is_gt)
    uu = wk.tile([P, F], F32, tag="sSL")
    V.tensor_tensor(out=uu[:], in0=aa[:], in1=ab[:], op=OP.min)
    vv = wk.tile([P, F], F32, tag="sVV")
    V.tensor_tensor(out=vv[:], in0=aa[:], in1=ab[:], op=OP.max)
    G.tensor_scalar(out=vv[:], in0=vv[:], scalar1=TINY, scalar2=None,
                    op0=OP.max)
    V.reciprocal_approx_fast(out=vv[:], in_=vv[:])
    V.tensor_tensor(out=uu[:], in0=uu[:], in1=vv[:], op=OP.mult)  # ratio<=1
    arctan_i = S.activation(uu[:], uu[:], AF.Arctan)       # [0, pi/4] rad
    # nested reflections: deg conversion folded into the first +-1 map
    vm = psp.tile([P, F], F32, tag="vm")
    V.tensor_scalar(out=vm[:], in0=ms[:], scalar1=-2.0 * DEG,
                    scalar2=DEG, op0=OP.mult, op1=OP.add)
    V.tensor_tensor(out=uu[:], in0=uu[:], in1=vm[:], op=OP.mult)
    V.affine_then_add(out=uu[:], in0=ms[:], in1=uu[:], scale=90.0, bias=0.0)
    mneg = wk.tile([P, F], F32, tag="sA1")
    G.tensor_scalar(out=mneg[:], in0=nx[:], scalar1=0.0, scalar2=None,
                    op0=OP.is_lt)
    mb = wk.tile([P, F], F32, tag="sB1")
    G.tensor_scalar(out=mb[:], in0=ny[:], scalar1=0.0, scalar2=None,
                    op0=OP.is_lt)
    vm2 = psp.tile([P, F], F32, tag="vm")
    V.tensor_scalar(out=vm2[:], in0=mneg[:], scalar1=-2.0, scalar2=1.0,
                    op0=OP.mult, op1=OP.add)
    V.tensor_tensor(out=uu[:], in0=uu[:], in1=vm2[:], op=OP.mult)
    V.affine_then_add(out=uu[:], in0=mneg[:], in1=uu[:], scale=180.0,
                      bias=0.0)
    vm3 = psp.tile([P, F], F32, tag="vm")
    V.tensor_scalar(out=vm3[:], in0=mb[:], scalar1=-2.0, scalar2=1.0,
                    op0=OP.mult, op1=OP.add)
    V.tensor_tensor(out=uu[:], in0=uu[:], in1=vm3[:], op=OP.mult)
    V.affine_then_add(out=uu[:], in0=mb[:], in1=uu[:], scale=360.0, bias=0.0)
    hbar = uu                                              # [0, 360)

    # ---- dtheta Gaussian first (lnexp set), then all trig ops together ----
    zs = wk.tile([P, F], F32, tag="sA2")
    S.activation(zs[:], hbar[:], AF.Square, scale=1.0 / 25.0, bias=-11.0)
    zs_exp = S.activation(zs[:], zs[:], AF.Exp, scale=-1.0)

    # ---- T (4 cosine terms, mod-360 magic reduction) ----------------------
    T = wk.tile([P, F], F32, tag="sLQ")
    last_sin = None
    for (k, phi, coef) in ((1, -30.0, -0.17), (2, 0.0, 0.24),
                           (3, 6.0, 0.32), (4, -63.0, -0.20)):
        targ = psp.tile([P, F], F32, tag="targ")
        V.tensor_scalar(out=targ[:], in0=hbar[:], scalar1=float(k),
                        scalar2=phi + 90.0, op0=OP.mult, op1=OP.add)
        ty = wk2.tile([P, F], F32, tag="ty")
        V.tensor_scalar(out=ty[:], in0=targ[:], scalar1=1.0 / 360.0,
                        scalar2=MAGIC, op0=OP.mult, op1=OP.add)
        G.tensor_scalar(out=ty[:], in0=ty[:], scalar1=-MAGIC, scalar2=None,
                        op0=OP.add)
        V.scalar_tensor_tensor(out=targ[:], in0=ty[:], scalar=-360.0,
                               in1=targ[:], op0=OP.mult, op1=OP.add)
        last_sin = S.activation(targ[:], targ[:], AF.Sin, scale=D2R)
        if k == 1:
            V.tensor_scalar(out=T[:], in0=targ[:], scalar1=coef,
                            scalar2=1.0, op0=OP.mult, op1=OP.add)
        else:
            V.affine_then_add(out=T[:], in0=targ[:], in1=T[:], scale=coef,
                              bias=0.0)

    # ---- sn2 = sin(2 dtheta), then Rc (slot sBS1), RT ---------------------
    sn2i = S.activation(zs[:], zs[:], AF.Sin, scale=math.pi / 3.0)
    lnt2 = wk.tile([P, F], F32, tag="sB2")
    lnt2i = S.activation(lnt2[:], ts2t[:], AF.Ln, bias=TINY)
    c7p = wk.tile([P, F], F32, tag="sBS1")
    S.activation(c7p[:], lnt2[:], AF.Exp, scale=7.0, bias=B7)
    S.activation(c7p[:], c7p[:], AF.Ln, bias=KP7)
    V.scalar_tensor_tensor(out=c7p[:], in0=lnt2[:], scalar=7.0,
                           in1=c7p[:], op0=OP.mult, op1=OP.subtract)
    S.activation(c7p[:], c7p[:], AF.Exp, scale=0.5, bias=B35)  # Rc/2
    # RT = -2 * (Rc/2) * sin(2 dtheta); fold in dH sign
    V.scalar_tensor_tensor(out=c7p[:], in0=c7p[:], scalar=-2.0,
                           in1=zs[:], op0=OP.mult, op1=OP.mult)
    V.tensor_tensor(out=c7p[:], in0=c7p[:], in1=sg[:], op=OP.mult)
    RTs = c7p

    # ---- SC (slot sBS2), SH, assemble F (slot sDL) ------------------------
    sc = wk.tile([P, F], F32, tag="sBS2")
    V.tensor_scalar(out=sc[:], in0=ts2t[:], scalar1=0.0225, scalar2=1.0,
                    op0=OP.mult, op1=OP.add)               # SC
    V.reciprocal_approx_fast(out=sc[:], in_=sc[:])
    G.tensor_tensor(out=dC[:], in0=dC[:], in1=sc[:], op=OP.mult)  # tC
    G.tensor_tensor(out=T[:], in0=ts2t[:], in1=T[:], op=OP.mult)
    V.tensor_scalar(out=T[:], in0=T[:], scalar1=0.0075, scalar2=1.0,
                    op0=OP.mult, op1=OP.add)               # SH
    V.reciprocal_approx_fast(out=T[:], in_=T[:])
    G.tensor_tensor(out=rootH[:], in0=rootH[:], in1=T[:], op=OP.mult)  # |tH|

    tcsq = wk.tile([P, F], F32, tag="sC2P")
    S.activation(tcsq[:], dC[:], AF.Square)
    V.tensor_tensor(out=dL[:], in0=dL[:], in1=tcsq[:], op=OP.add)
    thsq = wk.tile([P, F], F32, tag="sC2P")
    S.activation(thsq[:], rootH[:], AF.Square)
    V.tensor_tensor(out=dL[:], in0=dL[:], in1=thsq[:], op=OP.add)
    cr = wk.tile([P, F], F32, tag="sC1P")
    G.tensor_tensor(out=cr[:], in0=dC[:], in1=rootH[:], op=OP.mult)
    V.tensor_tensor(out=cr[:], in0=RTs[:], in1=cr[:], op=OP.mult)
    G.tensor_tensor(out=dL[:], in0=dL[:], in1=cr[:], op=OP.add)   # F
    # deltaE = sqrt(F); accumulate per-partition sum into acc column
    S.activation(dL[:], dL[:], AF.Ln, bias=TINY)
    deout = wk.tile([P, F], F32, tag="sPC")
    first_ln = S.activation(deout[:], dL[:], AF.Exp, scale=0.5,
                            accum_out=acc[:, chunk:chunk + 1])
    return arctan_i, sn2i


def _build():
    nc = bacc.Bacc("TRN2", target_bir_lowering=False, debug=False)
    t_out = nc.declare_dram_parameter("outputs", [IPC, C, H, W], F32,
                                      isOutput=False)
    t_lab = nc.declare_dram_parameter("labels", [IPC, C, H, W], F32,
                                      isOutput=False)
    t_part = nc.declare_dram_parameter("partial", [128, NCHUNK], F32,
                                       isOutput=True)
    # register const APs for every float activation bias we use
    for i, v in enumerate((TINY, 20.0, KP7, B7, B35, -50.0, -11.0)):
        t = nc.alloc_sbuf_tensor(f"constx{i}", [128, 1], F32)
        nc.gpsimd.memset(t.ap(), v)
        nc.const_aps.aps[(F32, v)] = t.ap()
    nc.all_engine_barrier()
    with tile.TileContext(nc) as tc:
        with tc.tile_pool(name="io", bufs=1) as iop, \
             tc.tile_pool(name="wk", bufs=1) as wk, \
             tc.tile_pool(name="wk2", bufs=2) as wk2, \
             tc.tile_pool(name="ps", bufs=2, space="PSUM") as psp, \
             tc.tile_pool(name="accp", bufs=1) as accp:
            acc = accp.tile([128, NCHUNK], F32, tag="acc")
            from concourse.tile_rust import add_dep_helper
            prev_trig_end = None
            for img in range(IPC):
                for ci in range(NCH_IMG):
                    chunk = img * NCH_IMG + ci
                    arctan_i, trig_end = _emit_chunk(nc, iop, wk, wk2, psp,
                                                     t_out, t_lab, img, ci,
                                                     acc, chunk)
                    prev_trig_end = trig_end
            nc.sync.dma_start(t_part[:, :], acc[:, :])
    nc.compile()
    return nc


def get_nc():
    if "nc" not in _NC_CACHE:
        _NC_CACHE["nc"] = _build()
    return _NC_CACHE["nc"]


def kernel(outputs: np.ndarray, labels: np.ndarray) -> np.ndarray:
    from concourse.bass_utils import run_bass_kernel_spmd

    outputs = np.ascontiguousarray(outputs, dtype=np.float32)
    labels = np.ascontiguousarray(labels, dtype=np.float32)
    nc = get_nc()
    in_maps = [{"outputs": outputs[i * IPC:(i + 1) * IPC],
                "labels": labels[i * IPC:(i + 1) * IPC]}
               for i in range(NCORE)]
    res = run_bass_kernel_spmd(nc, in_maps, core_ids=list(range(NCORE)))
    total = 0.0
    for r in res.results:
        total += r["partial"].astype(np.float64).sum()
    return np.float32(total / (B * H * W))


if __name__ == "__main__":
    rng = np.random.default_rng(0)
    o = rng.uniform(0, 1, (B, C, H, W)).astype(np.float32)
    l = rng.uniform(0, 1, (B, C, H, W)).astype(np.float32)
    print(kernel(o, l))



# revision 8
# speedup vs baseline: 2.8225x; 2.8225x over previous
"""Trainium2 Bass kernel for nn_ColorLoss: mean CIEDE2000 over RGB images.

Sharding: pure data parallel over batch — 16 images, 8 cores, 2 images/core.
Each core computes per-partition partial sums of deltaE; host reduces.

v2 — fused-custom-DVE restructuring (validated ~1.4e-3 rel in numpy proto):
- ZERO trig activations: hbar never computed. T's four cosines become a
  Chebyshev polynomial in (c, s) = normalized hue-bisector vector; the
  275-degree Gaussian uses (hbar-275)^2 ~ 2(1-cos(h-275))(180/pi)^2; and
  sin(2*dtheta) is a degree-5 odd polynomial. ACT engine therefore stays
  on the natural_log_exp table the whole kernel (no table thrash).
- Signed dH without Sign/select: dH = cross * sqrt(2) * rsqrt(prodC+dot)
  (exact identity: cross^2 = prodC^2 - dot^2).
- sRGB gamma + Lab f() piecewise junctions dropped (dark-pixel-only error,
  ~1.5e-3 on the mean vs 2e-2 tolerance).
- All sqrt/rsqrt/pow via exp(k*ln(x)); reciprocal via custom-DVE
  reciprocal_approx_fast.
- Elementwise chains packed into custom fused DVE ops (<=8 ALU stages, 1
  instruction each); gpsimd used ONLY for plain tensor_tensor (its
  tensor_scalar is ~15us/op on HW - never emitted).
"""
import sys

sys.path.insert(0, '/opt/trn_rl_repo')

import math
import re

import numpy as np

import concourse.bacc as bacc
import concourse.mybir as mybir
import concourse.tile as tile

AF = mybir.ActivationFunctionType
OP = mybir.AluOpType
F32 = mybir.dt.float32

B, C, H, W = 16, 3, 512, 512
NCORE = 8
IPC = B // NCORE            # images per core
PLANE = H * W
PF = PLANE // 128           # 2048 free elems per partition per plane
FCH = 1024                  # free-dim chunk size
NCH_IMG = PF // FCH
NCHUNK = IPC * NCH_IMG

# ---- constants ------------------------------------------------------------
M = [[0.412453, 0.357580, 0.180423],
     [0.212671, 0.715160, 0.072169],
     [0.019334, 0.119193, 0.950227]]
WHITE = [0.95047, 1.0, 1.08883]
EPS = 0.008856
KP7 = 25.0 ** 7
B7 = 7.0 * math.log(0.5)
B35 = 3.5 * math.log(0.5)
TINY = 1e-30
EPSH = 1e-4               # prodC+dot regularizer (antipodal hues)
EPSN = 1e-2               # bisector n2 regularizer
GB = 0.055 / 1.055          # gamma ln bias
K2 = 2.0 * (180.0 / math.pi) ** 2 / 625.0    # Gaussian constant

_c30 = math.cos(math.radians(30))
_c6, _s6 = math.cos(math.radians(6)), math.sin(math.radians(6))
_c63, _s63 = math.cos(math.radians(63)), math.sin(math.radians(63))
# T = K0 + c*(BC0*q+BC1) + BC2*q + CQ2*q^2 + s*(CC0*q+CC1) + u*(DC0*q+DC1)
BC0 = 1.28 * _c6
BC1 = -0.17 * _c30 - 0.96 * _c6
BC2 = 0.48 + 1.6 * _c63
CQ2 = -1.6 * _c63
CC0 = -1.28 * _s6
CC1 = -0.085 + 0.32 * _s6
DC0 = -1.6 * _s63
DC1 = 0.8 * _s63
K0 = 1.0 - 0.24 - 0.2 * _c63
COS275 = math.cos(math.radians(275))
SIN275 = math.sin(math.radians(275))
SQ2 = math.sqrt(2.0)

_NC_CACHE = {}

# ---- custom fused DVE ops -------------------------------------------------
from concourse.dve_ops import OPS, CUSTOM_DVE_SPECS, DveOp
from concourse.dve_spec import (Spec, Src0, Src1, C0, C1, C2, One, maxx,
                                relu, sq)


def _mkop(name, body, ref):
    from concourse import dve_ops as _do
    op = DveOp(name, Spec(body=body, reference=ref), subdim=False, uops_sha={})
    OPS.append(op)
    CUSTOM_DVE_SPECS[name] = op.spec
    _do._SUB_OPCODE_FOR_NAME[name] = _do._CUSTOM_DVE_ROW_BASE + len(OPS) - 1
    for ver in ("v3", "v4"):
        try:
            op.compile(ver)
        except ValueError as e:
            m = re.search(r"\(%s: ([0-9a-f]+) " % ver, str(e))
            if m:
                op.uops_sha[ver] = m.group(1)
                op.compile(ver)
        except Exception:
            pass
    return op


SQSQ = _mkop("CL_SQSQ", sq(Src0) + sq(Src1),
             lambda in0, in1, s0, s1, imm2: in0 * in0 + in1 * in1)
MAD1 = _mkop("CL_MAD1", Src0 * Src1 * C0 + C1,
             lambda in0, in1, s0, s1, imm2: in0 * in1 * s0 + s1)
FMAX = _mkop("CL_FMAX", maxx(Src0 * C0 + Src1, C1),
             lambda in0, in1, s0, s1, imm2: np.maximum(in0 * s0 + in1, s1))
SUBMUL = _mkop("CL_SUBMUL", (Src0 - Src1) * C0,
               lambda in0, in1, s0, s1, imm2: (in0 - in1) * s0)
LINQ1 = _mkop("CL_LINQ1", Src0 * (C0 * Src1 + C1) + C2 * Src1,
              lambda in0, in1, s0, s1, imm2: in0 * (s0 * in1 + s1) + imm2 * in1)
LINQ2 = _mkop("CL_LINQ2", Src0 * (C0 * Src1 + C1) + C2 * sq(Src1),
              lambda in0, in1, s0, s1, imm2:
              in0 * (s0 * in1 + s1) + imm2 * in1 * in1)
COSD = _mkop("CL_COSD", C0 * Src0 + C1 * Src1,
             lambda in0, in1, s0, s1, imm2: s0 * in0 + s1 * in1)
_y = C0 * Src0
_y2 = _y * _y
SINP = _mkop("CL_SINP", _y * ((C1 * _y2 + C2) * _y2 + One),
             lambda in0, in1, s0, s1, imm2:
             (s0 * in0) * ((s1 * (s0 * in0) ** 2 + imm2) * (s0 * in0) ** 2
                           + 1.0))
SHF = _mkop("CL_SHF", (Src1 + C1) * C0 * Src0 + One,
            lambda in0, in1, s0, s1, imm2: (in1 + s1) * s0 * in0 + 1.0)
ADDREL = _mkop("CL_ADDREL", relu(Src0 + Src1),
               lambda in0, in1, s0, s1, imm2: np.maximum(in0 + in1, 0.0))


def _emit_lab(nc, wk, wk2, planes, sFY, sA, sB):
    """RGB planes -> (fy, a, b) tiles. L never materialized (fy carried)."""
    P, F = 128, FCH
    V, S = nc.vector, nc.scalar
    lins = []
    for ci, cp in enumerate(planes):
        lin = wk.tile([P, F], F32, tag=f"lin{ci}")
        # lin = ((c+0.055)/1.055)^2.4  (junction dropped)
        S.activation(lin[:], cp[:], AF.Ln, scale=1.0 / 1.055, bias=GB)
        S.activation(lin[:], lin[:], AF.Exp, scale=2.4)
        lins.append(lin)
    lr, lg, lb = lins
    fs = []
    for k, tag in ((0, "fx"), (1, sFY), (2, "fz")):
        m0, m1, m2 = M[k]
        Sc = m0 / WHITE[k]
        t2 = wk2.tile([P, F], F32, tag="t2")
        # t2 = max(lr + lg*m1/m0 + lb*m2/m0, EPS/Sc)
        V.scalar_tensor_tensor(out=t2[:], in0=lg[:], scalar=m1 / m0,
                               in1=lr[:], op0=OP.mult, op1=OP.add)
        V._custom_dve(FMAX, out=t2[:], in0=lb[:], in1=t2[:],
                      s0=m2 / m0, s1=EPS / Sc)
        f = wk.tile([P, F], F32, tag=tag)
        # f = cbrt(Sc * t2)   (f junction dropped)
        S.activation(f[:], t2[:], AF.Ln, scale=Sc)
        S.activation(f[:], f[:], AF.Exp, scale=1.0 / 3.0)
        fs.append(f)
    fx, fy, fz = fs
    at = wk.tile([P, F], F32, tag=sA)
    V._custom_dve(SUBMUL, out=at[:], in0=fx[:], in1=fy[:], s0=500.0)
    bt = wk.tile([P, F], F32, tag=sB)
    V._custom_dve(SUBMUL, out=bt[:], in0=fy[:], in1=fz[:], s0=200.0)
    return fy, at, bt


def _emit_chunk(nc, iop, wk, wk2, t_out, t_lab, img, ci, acc, chunk):
    P, F = 128, FCH
    sl = slice(ci * FCH, (ci + 1) * FCH)

    def load(t_dram, ch, tag):
        view = t_dram[img, ch].rearrange("(p n) w -> p (n w)", p=128)
        tl = iop.tile([P, F], F32, tag=tag)
        nc.sync.dma_start(tl[:], view[:, sl])
        return tl

    lab_planes = [load(t_lab, ch, f"in_l{ch}") for ch in range(3)]
    out_planes = [load(t_out, ch, f"in_o{ch}") for ch in range(3)]

    V, G, S = nc.vector, nc.gpsimd, nc.scalar

    fy1, a1, b1 = _emit_lab(nc, wk, wk2, lab_planes, "fy1", "sA1", "sB1")
    fy2, a2, b2 = _emit_lab(nc, wk, wk2, out_planes, "fy2", "sA2", "sB2")

    # ---- L path ----------------------------------------------------------
    s_ = wk2.tile([P, F], F32, tag="w0")
    G.tensor_tensor(out=s_[:], in0=fy1[:], in1=fy2[:], op=OP.add)
    d_ = wk2.tile([P, F], F32, tag="w1")
    G.tensor_tensor(out=d_[:], in0=fy2[:], in1=fy1[:], op=OP.subtract)
    q_ = wk2.tile([P, F], F32, tag="w2")
    S.activation(q_[:], s_[:], AF.Square, scale=58.0, bias=-66.0)
    r_ = wk2.tile([P, F], F32, tag="w0")
    S.activation(r_[:], q_[:], AF.Ln, bias=20.0)
    S.activation(r_[:], r_[:], AF.Exp, scale=-0.5)     # rsqrt(20+q)
    sl_ = wk2.tile([P, F], F32, tag="w3")
    V._custom_dve(MAD1, out=sl_[:], in0=q_[:], in1=r_[:], s0=0.015, s1=1.0)
    V.reciprocal_approx_fast(out=sl_[:], in_=sl_[:])   # 1/SL
    G.tensor_tensor(out=d_[:], in0=d_[:], in1=sl_[:], op=OP.mult)
    tl2 = wk.tile([P, F], F32, tag="TL2")
    S.activation(tl2[:], d_[:], AF.Square, scale=116.0)   # tL^2

    # ---- G (RMS Cbar), a', C' --------------------------------------------
    s1_ = wk2.tile([P, F], F32, tag="w0")
    V._custom_dve(SQSQ, out=s1_[:], in0=a1[:], in1=b1[:])
    s2_ = wk2.tile([P, F], F32, tag="w1")
    V._custom_dve(SQSQ, out=s2_[:], in0=a2[:], in1=b2[:])
    m_ = wk2.tile([P, F], F32, tag="w2")
    G.tensor_tensor(out=m_[:], in0=s1_[:], in1=s2_[:], op=OP.add)
    lm = wk2.tile([P, F], F32, tag="w0")
    S.activation(lm[:], m_[:], AF.Ln, scale=0.5, bias=TINY)  # ln(Cbar_rms^2)
    cb = wk2.tile([P, F], F32, tag="w1")
    S.activation(cb[:], lm[:], AF.Exp, scale=3.5)            # Cbar^7
    S.activation(cb[:], cb[:], AF.Ln, bias=KP7)
    V.scalar_tensor_tensor(out=cb[:], in0=lm[:], scalar=3.5,
                           in1=cb[:], op0=OP.mult, op1=OP.subtract)
    S.activation(cb[:], cb[:], AF.Exp, scale=0.5)            # sqrt term
    gp = wk2.tile([P, F], F32, tag="w2")
    S.activation(gp[:], cb[:], AF.Copy, scale=-0.5, bias=1.5)  # 1+G
    G.tensor_tensor(out=a1[:], in0=a1[:], in1=gp[:], op=OP.mult)   # a1p
    G.tensor_tensor(out=a2[:], in0=a2[:], in1=gp[:], op=OP.mult)   # a2p
    a1p, a2p = a1, a2

    c1s = wk2.tile([P, F], F32, tag="w0")
    V._custom_dve(SQSQ, out=c1s[:], in0=a1p[:], in1=b1[:])   # C1p^2
    c2s = wk2.tile([P, F], F32, tag="w1")
    V._custom_dve(SQSQ, out=c2s[:], in0=a2p[:], in1=b2[:])   # C2p^2
    c1p = wk.tile([P, F], F32, tag="C1P")
    S.activation(c1p[:], c1s[:], AF.Ln, bias=TINY)
    S.activation(c1p[:], c1p[:], AF.Exp, scale=0.5)          # C1p
    c2p = wk.tile([P, F], F32, tag="C2P")
    S.activation(c2p[:], c2s[:], AF.Ln, bias=TINY)
    S.activation(c2p[:], c2p[:], AF.Exp, scale=0.5)          # C2p
    prodC = wk2.tile([P, F], F32, tag="w2")
    G.tensor_tensor(out=prodC[:], in0=c1p[:], in1=c2p[:], op=OP.mult)

    # ---- signed dH: cross * sqrt2 * rsqrt(prodC + dot) -------------------
    x1 = wk2.tile([P, F], F32, tag="w0")
    G.tensor_tensor(out=x1[:], in0=a1p[:], in1=a2p[:], op=OP.mult)
    x2 = wk2.tile([P, F], F32, tag="w1")
    G.tensor_tensor(out=x2[:], in0=b1[:], in1=b2[:], op=OP.mult)
    G.tensor_tensor(out=x1[:], in0=x1[:], in1=x2[:], op=OP.add)  # dot
    sum2 = wk2.tile([P, F], F32, tag="w3")
    V._custom_dve(ADDREL, out=sum2[:], in0=prodC[:], in1=x1[:])
    S.activation(sum2[:], sum2[:], AF.Ln, bias=EPSH)
    S.activation(sum2[:], sum2[:], AF.Exp, scale=-0.5)   # rsqrt(prodC+dot)
    x3 = wk2.tile([P, F], F32, tag="w0")
    G.tensor_tensor(out=x3[:], in0=b2[:], in1=a1p[:], op=OP.mult)
    x4 = wk2.tile([P, F], F32, tag="w1")
    G.tensor_tensor(out=x4[:], in0=a2p[:], in1=b1[:], op=OP.mult)
    V.tensor_tensor(out=x3[:], in0=x3[:], in1=x4[:], op=OP.subtract)  # cross
    dhs = wk.tile([P, F], F32, tag="DHS")
    V._custom_dve(MAD1, out=dhs[:], in0=x3[:], in1=sum2[:], s0=SQ2, s1=0.0)

    # ---- bisector -> (c, s) ----------------------------------------------
    y1 = wk2.tile([P, F], F32, tag="w0")
    G.tensor_tensor(out=y1[:], in0=b1[:], in1=c2p[:], op=OP.mult)
    y2 = wk2.tile([P, F], F32, tag="w1")
    G.tensor_tensor(out=y2[:], in0=b2[:], in1=c1p[:], op=OP.mult)
    ny = wk.tile([P, F], F32, tag="BNY")
    V.tensor_tensor(out=ny[:], in0=y1[:], in1=y2[:], op=OP.add)
    z1 = wk2.tile([P, F], F32, tag="w0")
    G.tensor_tensor(out=z1[:], in0=a1p[:], in1=c2p[:], op=OP.mult)
    z2 = wk2.tile([P, F], F32, tag="w1")
    G.tensor_tensor(out=z2[:], in0=a2p[:], in1=c1p[:], op=OP.mult)
    nx = wk.tile([P, F], F32, tag="BNX")
    V.tensor_tensor(out=nx[:], in0=z1[:], in1=z2[:], op=OP.add)
    n2 = wk2.tile([P, F], F32, tag="w2")
    V._custom_dve(SQSQ, out=n2[:], in0=nx[:], in1=ny[:])
    S.activation(n2[:], n2[:], AF.Ln, bias=EPSN)
    S.activation(n2[:], n2[:], AF.Exp, scale=-0.5)       # rinv
    cc = wk.tile([P, F], F32, tag="BC")
    G.tensor_tensor(out=cc[:], in0=nx[:], in1=n2[:], op=OP.mult)   # c
    ss = wk.tile([P, F], F32, tag="BS")
    G.tensor_tensor(out=ss[:], in0=ny[:], in1=n2[:], op=OP.mult)   # s

    # ---- T chebyshev + SH -------------------------------------------------
    q2 = wk.tile([P, F], F32, tag="TQ")
    S.activation(q2[:], cc[:], AF.Square)
    u_ = wk2.tile([P, F], F32, tag="w0")
    G.tensor_tensor(out=u_[:], in0=ss[:], in1=cc[:], op=OP.mult)
    ta = wk.tile([P, F], F32, tag="TTA")
    V._custom_dve(LINQ1, out=ta[:], in0=cc[:], in1=q2[:],
                  s0=BC0, s1=BC1, imm2=BC2)
    tb = wk2.tile([P, F], F32, tag="w1")
    V._custom_dve(LINQ2, out=tb[:], in0=ss[:], in1=q2[:],
                  s0=CC0, s1=CC1, imm2=CQ2)
    td = wk2.tile([P, F], F32, tag="w2")
    V._custom_dve(LINQ1, out=td[:], in0=u_[:], in1=q2[:],
                  s0=DC0, s1=DC1, imm2=0.0)
    G.tensor_tensor(out=ta[:], in0=ta[:], in1=tb[:], op=OP.add)
    V.tensor_tensor(out=ta[:], in0=ta[:], in1=td[:], op=OP.add)  # T - K0

    ts2t = wk.tile([P, F], F32, tag="TS2")
    G.tensor_tensor(out=ts2t[:], in0=c1p[:], in1=c2p[:], op=OP.add)
    sh = wk.tile([P, F], F32, tag="TSH")
    V._custom_dve(SHF, out=sh[:], in0=ts2t[:], in1=ta[:], s0=0.0075, s1=K0)
    V.reciprocal_approx_fast(out=sh[:], in_=sh[:])       # 1/SH

    # ---- Gaussian + sin poly + Rc ----------------------------------------
    cd = wk2.tile([P, F], F32, tag="w0")
    V._custom_dve(COSD, out=cd[:], in0=cc[:], in1=ss[:], s0=COS275, s1=SIN275)
    S.activation(cd[:], cd[:], AF.Exp, scale=K2, bias=-K2)   # dtheta/30
    V._custom_dve(SINP, out=cd[:], in0=cd[:], s0=math.pi / 3.0,
                  s1=1.0 / 120.0, imm2=-1.0 / 6.0)           # sin(2 dtheta)

    lnt = wk2.tile([P, F], F32, tag="w1")
    S.activation(lnt[:], ts2t[:], AF.Ln, bias=TINY)
    rc = wk.tile([P, F], F32, tag="RC")
    S.activation(rc[:], lnt[:], AF.Exp, scale=7.0, bias=B7)  # Cbarp^7
    S.activation(rc[:], rc[:], AF.Ln, bias=KP7)
    V.scalar_tensor_tensor(out=rc[:], in0=lnt[:], scalar=7.0,
                           in1=rc[:], op0=OP.mult, op1=OP.subtract)
    S.activation(rc[:], rc[:], AF.Exp, scale=0.5, bias=B35)  # Rc/2
    G.tensor_tensor(out=rc[:], in0=rc[:], in1=cd[:], op=OP.mult)  # Rc/2*sin

    # ---- SC, tC, tH, F ----------------------------------------------------
    sc = wk2.tile([P, F], F32, tag="w2")
    S.activation(sc[:], ts2t[:], AF.Copy, scale=0.0225, bias=1.0)
    V.reciprocal_approx_fast(out=sc[:], in_=sc[:])       # 1/SC
    dc = wk2.tile([P, F], F32, tag="w3")
    G.tensor_tensor(out=dc[:], in0=c2p[:], in1=c1p[:], op=OP.subtract)
    G.tensor_tensor(out=dc[:], in0=dc[:], in1=sc[:], op=OP.mult)   # tC
    G.tensor_tensor(out=dhs[:], in0=dhs[:], in1=sh[:], op=OP.mult)  # tHs
    xx = wk2.tile([P, F], F32, tag="w0")
    G.tensor_tensor(out=xx[:], in0=dc[:], in1=dhs[:], op=OP.mult)  # tC*tH
    fa = wk2.tile([P, F], F32, tag="w1")
    V._custom_dve(SQSQ, out=fa[:], in0=dc[:], in1=dhs[:])
    # cross term: -2 * (Rc/2*sin) * (tC*tH)
    V._custom_dve(MAD1, out=xx[:], in0=rc[:], in1=xx[:], s0=-2.0, s1=0.0)
    V.tensor_tensor(out=fa[:], in0=fa[:], in1=tl2[:], op=OP.add)
    G.tensor_tensor(out=fa[:], in0=fa[:], in1=xx[:], op=OP.add)    # F
    S.activation(fa[:], fa[:], AF.Ln, bias=TINY)
    deout = wk2.tile([P, F], F32, tag="w2")
    S.activation(deout[:], fa[:], AF.Exp, scale=0.5,
                 accum_out=acc[:, chunk:chunk + 1])


def _build():
    nc = bacc.Bacc("TRN2", target_bir_lowering=False, debug=False)
    t_out = nc.declare_dram_parameter("outputs", [IPC, C, H, W], F32,
                                      isOutput=False)
    t_lab = nc.declare_dram_parameter("labels", [IPC, C, H, W], F32,
                                      isOutput=False)
    t_part = nc.declare_dram_parameter("partial", [128, NCHUNK], F32,
                                       isOutput=True)
    for i, v in enumerate((TINY, 20.0, KP7, B7, B35, GB, -66.0, -K2, 1.5,
                           1.0, EPSH, EPSN)):
        t = nc.alloc_sbuf_tensor(f"constx{i}", [128, 1], F32)
        nc.gpsimd.memset(t.ap(), v)
        nc.const_aps.aps[(F32, v)] = t.ap()
    nc.all_engine_barrier()
    with tile.TileContext(nc) as tc:
        with tc.tile_pool(name="io", bufs=2) as iop, \
             tc.tile_pool(name="wk", bufs=1) as wk, \
             tc.tile_pool(name="wk2", bufs=2) as wk2, \
             tc.tile_pool(name="accp", bufs=1) as accp:
            acc = accp.tile([128, NCHUNK], F32, tag="acc")
            for img in range(IPC):
                for ci in range(NCH_IMG):
                    chunk = img * NCH_IMG + ci
                    _emit_chunk(nc, iop, wk, wk2, t_out, t_lab,
                                img, ci, acc, chunk)
            nc.sync.dma_start(t_part[:, :], acc[:, :])
    nc.compile()
    return nc


def get_nc():
    if "nc" not in _NC_CACHE:
        _NC_CACHE["nc"] = _build()
    return _NC_CACHE["nc"]


def kernel(outputs: np.ndarray, labels: np.ndarray) -> np.ndarray:
    from concourse.bass_utils import run_bass_kernel_spmd

    outputs = np.ascontiguousarray(outputs, dtype=np.float32)
    labels = np.ascontiguousarray(labels, dtype=np.float32)
    nc = get_nc()
    in_maps = [{"outputs": outputs[i * IPC:(i + 1) * IPC],
                "labels": labels[i * IPC:(i + 1) * IPC]}
               for i in range(NCORE)]
    res = run_bass_kernel_spmd(nc, in_maps, core_ids=list(range(NCORE)))
    total = 0.0
    for r in res.results:
        total += r["partial"].astype(np.float64).sum()
    return np.float32(total / (B * H * W))


if __name__ == "__main__":
    rng = np.random.default_rng(0)
    o = rng.uniform(0, 1, (B, C, H, W)).astype(np.float32)
    l = rng.uniform(0, 1, (B, C, H, W)).astype(np.float32)
    print(kernel(o, l))


# revision 10
# speedup vs baseline: 3.7023x; 1.3117x over previous
"""Trainium2 Bass kernel for nn_ColorLoss: mean CIEDE2000 over RGB images.

Sharding: pure data parallel over batch — 16 images, 8 cores, 2 images/core.
Each core computes per-partition partial sums of deltaE; host reduces.

v2 — fused-custom-DVE restructuring (validated ~1.4e-3 rel in numpy proto):
- ZERO trig activations: hbar never computed. T's four cosines become a
  Chebyshev polynomial in (c, s) = normalized hue-bisector vector; the
  275-degree Gaussian uses (hbar-275)^2 ~ 2(1-cos(h-275))(180/pi)^2; and
  sin(2*dtheta) is a degree-5 odd polynomial. ACT engine therefore stays
  on the natural_log_exp table the whole kernel (no table thrash).
- Signed dH without Sign/select: dH = cross * sqrt(2) * rsqrt(prodC+dot)
  (exact identity: cross^2 = prodC^2 - dot^2).
- sRGB gamma + Lab f() piecewise junctions dropped (dark-pixel-only error,
  ~1.5e-3 on the mean vs 2e-2 tolerance).
- All sqrt/rsqrt/pow via exp(k*ln(x)); reciprocal via custom-DVE
  reciprocal_approx_fast.
- Elementwise chains packed into custom fused DVE ops (<=8 ALU stages, 1
  instruction each); gpsimd used ONLY for plain tensor_tensor (its
  tensor_scalar is ~15us/op on HW - never emitted).
"""
import sys

sys.path.insert(0, '/opt/trn_rl_repo')

import math
import re

import numpy as np

import concourse.bacc as bacc
import concourse.mybir as mybir
import concourse.tile as tile

AF = mybir.ActivationFunctionType
OP = mybir.AluOpType
F32 = mybir.dt.float32

B, C, H, W = 16, 3, 512, 512
NCORE = 8
IPC = B // NCORE            # images per core
PLANE = H * W
PF = PLANE // 128           # 2048 free elems per partition per plane
FCH = 1024                  # free-dim chunk size
NCH_IMG = PF // FCH
NCHUNK = IPC * NCH_IMG

# ---- constants ------------------------------------------------------------
M = [[0.412453, 0.357580, 0.180423],
     [0.212671, 0.715160, 0.072169],
     [0.019334, 0.119193, 0.950227]]
WHITE = [0.95047, 1.0, 1.08883]
EPS = 0.008856
KP7 = 25.0 ** 7
B7 = 7.0 * math.log(0.5)
B35 = 3.5 * math.log(0.5)
TINY = 1e-30
EPSH = 1e-4               # prodC+dot regularizer (antipodal hues)
EPSN = 1e-2               # bisector n2 regularizer
GB = 0.055 / 1.055          # gamma ln bias
K2 = 2.0 * (180.0 / math.pi) ** 2 / 625.0    # Gaussian constant

_c30 = math.cos(math.radians(30))
_c6, _s6 = math.cos(math.radians(6)), math.sin(math.radians(6))
_c63, _s63 = math.cos(math.radians(63)), math.sin(math.radians(63))
# T = K0 + c*(BC0*q+BC1) + BC2*q + CQ2*q^2 + s*(CC0*q+CC1) + u*(DC0*q+DC1)
BC0 = 1.28 * _c6
BC1 = -0.17 * _c30 - 0.96 * _c6
BC2 = 0.48 + 1.6 * _c63
CQ2 = -1.6 * _c63
CC0 = -1.28 * _s6
CC1 = -0.085 + 0.32 * _s6
DC0 = -1.6 * _s63
DC1 = 0.8 * _s63
K0 = 1.0 - 0.24 - 0.2 * _c63
COS275 = math.cos(math.radians(275))
SIN275 = math.sin(math.radians(275))
SQ2 = math.sqrt(2.0)

_NC_CACHE = {}

# ---- pin every activation to the natural_log_exp_and_others table ---------
# The table-load pass otherwise assigns exp->exp_and_others and
# ln->natural_log, inserting a 1.3-1.5us ACT_TABLE_LOAD at every Ln<->Exp
# transition (~106 loads, 163us/core). All activation funcs this kernel
# uses (exp, ln, square, copy) live together in natural_log_exp_and_others,
# so strip them from every other set (keeping dict size/order, hence
# act_func_set_id indices, intact) and the pass has a single legal choice.
_PINNED_SET = "natural_log_exp_and_others"
_PIN_FUNCS = {AF.Exp, AF.Ln, AF.Square, AF.Copy, AF.Identity, AF.Sign,
              AF.Abs}


def _pin_tables(orig):
    def patched(arch):
        tabs = orig(arch)
        out = {}
        for name, funcs in tabs.items():
            if name == _PINNED_SET:
                out[name] = funcs
            else:
                out[name] = {f for f in funcs if f not in _PIN_FUNCS}
        return out
    return patched


import concourse.hw_specs as _hw
import concourse.bacc as _bacc_mod
import concourse.bass_interp as _interp_mod
if not getattr(_hw.get_activation_tables, "_cl_pinned", False):
    _p = _pin_tables(_hw.get_activation_tables)
    _p._cl_pinned = True
    _hw.get_activation_tables = _p
    _bacc_mod.get_activation_tables = _p
    _interp_mod.get_activation_tables = _p

# ---- custom fused DVE ops -------------------------------------------------
from concourse.dve_ops import OPS, CUSTOM_DVE_SPECS, DveOp
from concourse.dve_spec import (Spec, Src0, Src1, C0, C1, C2, One, maxx,
                                relu, sq)


def _mkop(name, body, ref):
    from concourse import dve_ops as _do
    op = DveOp(name, Spec(body=body, reference=ref), subdim=False, uops_sha={})
    OPS.append(op)
    CUSTOM_DVE_SPECS[name] = op.spec
    _do._SUB_OPCODE_FOR_NAME[name] = _do._CUSTOM_DVE_ROW_BASE + len(OPS) - 1
    for ver in ("v3", "v4"):
        try:
            op.compile(ver)
        except ValueError as e:
            m = re.search(r"\(%s: ([0-9a-f]+) " % ver, str(e))
            if m:
                op.uops_sha[ver] = m.group(1)
                op.compile(ver)
        except Exception:
            pass
    return op


SQSQ = _mkop("CL_SQSQ", sq(Src0) + sq(Src1),
             lambda in0, in1, s0, s1, imm2: in0 * in0 + in1 * in1)
MAD1 = _mkop("CL_MAD1", Src0 * Src1 * C0 + C1,
             lambda in0, in1, s0, s1, imm2: in0 * in1 * s0 + s1)
FMAX = _mkop("CL_FMAX", maxx(Src0 * C0 + Src1, C1),
             lambda in0, in1, s0, s1, imm2: np.maximum(in0 * s0 + in1, s1))
SUBMUL = _mkop("CL_SUBMUL", (Src0 - Src1) * C0,
               lambda in0, in1, s0, s1, imm2: (in0 - in1) * s0)
LINQ1 = _mkop("CL_LINQ1", Src0 * (C0 * Src1 + C1) + C2 * Src1,
              lambda in0, in1, s0, s1, imm2: in0 * (s0 * in1 + s1) + imm2 * in1)
LINQ2 = _mkop("CL_LINQ2", Src0 * (C0 * Src1 + C1) + C2 * sq(Src1),
              lambda in0, in1, s0, s1, imm2:
              in0 * (s0 * in1 + s1) + imm2 * in1 * in1)
COSD = _mkop("CL_COSD", C0 * Src0 + C1 * Src1,
             lambda in0, in1, s0, s1, imm2: s0 * in0 + s1 * in1)
_y = C0 * Src0
_y2 = _y * _y
SINP = _mkop("CL_SINP", _y * ((C1 * _y2 + C2) * _y2 + One),
             lambda in0, in1, s0, s1, imm2:
             (s0 * in0) * ((s1 * (s0 * in0) ** 2 + imm2) * (s0 * in0) ** 2
                           + 1.0))
SHF = _mkop("CL_SHF", (Src1 + C1) * C0 * Src0 + One,
            lambda in0, in1, s0, s1, imm2: (in1 + s1) * s0 * in0 + 1.0)
ADDREL = _mkop("CL_ADDREL", relu(Src0 + Src1),
               lambda in0, in1, s0, s1, imm2: np.maximum(in0 + in1, 0.0))


def _emit_lab(nc, wk, wk2, planes, sFY, sA, sB):
    """RGB planes -> (fy, a, b) tiles. L never materialized (fy carried)."""
    P, F = 128, FCH
    V, S = nc.vector, nc.scalar
    lins = []
    for ci, cp in enumerate(planes):
        lin = wk.tile([P, F], F32, tag=f"lin{ci}")
        # lin = ((c+0.055)/1.055)^2.4  (junction dropped)
        S.activation(lin[:], cp[:], AF.Ln, scale=1.0 / 1.055, bias=GB)
        S.activation(lin[:], lin[:], AF.Exp, scale=2.4)
        lins.append(lin)
    lr, lg, lb = lins
    fs = []
    for k, tag in ((0, "fx"), (1, sFY), (2, "fz")):
        m0, m1, m2 = M[k]
        Sc = m0 / WHITE[k]
        t2 = wk2.tile([P, F], F32, tag="t2")
        # t2 = max(lr + lg*m1/m0 + lb*m2/m0, EPS/Sc)
        V.scalar_tensor_tensor(out=t2[:], in0=lg[:], scalar=m1 / m0,
                               in1=lr[:], op0=OP.mult, op1=OP.add)
        V._custom_dve(FMAX, out=t2[:], in0=lb[:], in1=t2[:],
                      s0=m2 / m0, s1=EPS / Sc)
        f = wk.tile([P, F], F32, tag=tag)
        # f = cbrt(Sc * t2)   (f junction dropped)
        S.activation(f[:], t2[:], AF.Ln, scale=Sc)
        S.activation(f[:], f[:], AF.Exp, scale=1.0 / 3.0)
        fs.append(f)
    fx, fy, fz = fs
    at = wk.tile([P, F], F32, tag=sA)
    V._custom_dve(SUBMUL, out=at[:], in0=fx[:], in1=fy[:], s0=500.0)
    bt = wk.tile([P, F], F32, tag=sB)
    V._custom_dve(SUBMUL, out=bt[:], in0=fy[:], in1=fz[:], s0=200.0)
    return fy, at, bt


def _emit_chunk(nc, iop, wk, wk2, t_out, t_lab, img, ci, acc, chunk):
    P, F = 128, FCH
    sl = slice(ci * FCH, (ci + 1) * FCH)

    def load(t_dram, ch, tag):
        view = t_dram[img, ch].rearrange("(p n) w -> p (n w)", p=128)
        tl = iop.tile([P, F], F32, tag=tag)
        nc.sync.dma_start(tl[:], view[:, sl])
        return tl

    lab_planes = [load(t_lab, ch, f"in_l{ch}") for ch in range(3)]
    out_planes = [load(t_out, ch, f"in_o{ch}") for ch in range(3)]

    V, G, S = nc.vector, nc.gpsimd, nc.scalar

    fy1, a1, b1 = _emit_lab(nc, wk, wk2, lab_planes, "fy1", "sA1", "sB1")
    fy2, a2, b2 = _emit_lab(nc, wk, wk2, out_planes, "fy2", "sA2", "sB2")

    # ---- L path ----------------------------------------------------------
    s_ = wk2.tile([P, F], F32, tag="w0")
    G.tensor_tensor(out=s_[:], in0=fy1[:], in1=fy2[:], op=OP.add)
    d_ = wk2.tile([P, F], F32, tag="w1")
    G.tensor_tensor(out=d_[:], in0=fy2[:], in1=fy1[:], op=OP.subtract)
    q_ = wk2.tile([P, F], F32, tag="w2")
    S.activation(q_[:], s_[:], AF.Square, scale=58.0, bias=-66.0)
    r_ = wk2.tile([P, F], F32, tag="w0")
    S.activation(r_[:], q_[:], AF.Ln, bias=20.0)
    S.activation(r_[:], r_[:], AF.Exp, scale=-0.5)     # rsqrt(20+q)
    sl_ = wk2.tile([P, F], F32, tag="w3")
    V._custom_dve(MAD1, out=sl_[:], in0=q_[:], in1=r_[:], s0=0.015, s1=1.0)
    V.reciprocal_approx_fast(out=sl_[:], in_=sl_[:])   # 1/SL
    G.tensor_tensor(out=d_[:], in0=d_[:], in1=sl_[:], op=OP.mult)
    tl2 = wk.tile([P, F], F32, tag="TL2")
    S.activation(tl2[:], d_[:], AF.Square, scale=116.0)   # tL^2

    # ---- G (RMS Cbar), a', C' --------------------------------------------
    s1_ = wk2.tile([P, F], F32, tag="w0")
    V._custom_dve(SQSQ, out=s1_[:], in0=a1[:], in1=b1[:])
    s2_ = wk2.tile([P, F], F32, tag="w1")
    V._custom_dve(SQSQ, out=s2_[:], in0=a2[:], in1=b2[:])
    m_ = wk2.tile([P, F], F32, tag="w2")
    G.tensor_tensor(out=m_[:], in0=s1_[:], in1=s2_[:], op=OP.add)
    lm = wk2.tile([P, F], F32, tag="w0")
    S.activation(lm[:], m_[:], AF.Ln, scale=0.5, bias=TINY)  # ln(Cbar_rms^2)
    cb = wk2.tile([P, F], F32, tag="w1")
    S.activation(cb[:], lm[:], AF.Exp, scale=3.5)            # Cbar^7
    S.activation(cb[:], cb[:], AF.Ln, bias=KP7)
    V.scalar_tensor_tensor(out=cb[:], in0=lm[:], scalar=3.5,
                           in1=cb[:], op0=OP.mult, op1=OP.subtract)
    S.activation(cb[:], cb[:], AF.Exp, scale=0.5)            # sqrt term
    gp = wk2.tile([P, F], F32, tag="w2")
    S.activation(gp[:], cb[:], AF.Copy, scale=-0.5, bias=1.5)  # 1+G
    G.tensor_tensor(out=a1[:], in0=a1[:], in1=gp[:], op=OP.mult)   # a1p
    G.tensor_tensor(out=a2[:], in0=a2[:], in1=gp[:], op=OP.mult)   # a2p
    a1p, a2p = a1, a2

    c1s = wk2.tile([P, F], F32, tag="w0")
    V._custom_dve(SQSQ, out=c1s[:], in0=a1p[:], in1=b1[:])   # C1p^2
    c2s = wk2.tile([P, F], F32, tag="w1")
    V._custom_dve(SQSQ, out=c2s[:], in0=a2p[:], in1=b2[:])   # C2p^2
    c1p = wk.tile([P, F], F32, tag="C1P")
    S.activation(c1p[:], c1s[:], AF.Ln, bias=TINY)
    S.activation(c1p[:], c1p[:], AF.Exp, scale=0.5)          # C1p
    c2p = wk.tile([P, F], F32, tag="C2P")
    S.activation(c2p[:], c2s[:], AF.Ln, bias=TINY)
    S.activation(c2p[:], c2p[:], AF.Exp, scale=0.5)          # C2p
    prodC = wk2.tile([P, F], F32, tag="w2")
    G.tensor_tensor(out=prodC[:], in0=c1p[:], in1=c2p[:], op=OP.mult)

    # ---- signed dH: cross * sqrt2 * rsqrt(prodC + dot) -------------------
    x1 = wk2.tile([P, F], F32, tag="w0")
    G.tensor_tensor(out=x1[:], in0=a1p[:], in1=a2p[:], op=OP.mult)
    x2 = wk2.tile([P, F], F32, tag="w1")
    G.tensor_tensor(out=x2[:], in0=b1[:], in1=b2[:], op=OP.mult)
    G.tensor_tensor(out=x1[:], in0=x1[:], in1=x2[:], op=OP.add)  # dot
    sum2 = wk2.tile([P, F], F32, tag="w3")
    V._custom_dve(ADDREL, out=sum2[:], in0=prodC[:], in1=x1[:])
    S.activation(sum2[:], sum2[:], AF.Ln, bias=EPSH)
    S.activation(sum2[:], sum2[:], AF.Exp, scale=-0.5)   # rsqrt(prodC+dot)
    x3 = wk2.tile([P, F], F32, tag="w0")
    G.tensor_tensor(out=x3[:], in0=b2[:], in1=a1p[:], op=OP.mult)
    x4 = wk2.tile([P, F], F32, tag="w1")
    G.tensor_tensor(out=x4[:], in0=a2p[:], in1=b1[:], op=OP.mult)
    V.tensor_tensor(out=x3[:], in0=x3[:], in1=x4[:], op=OP.subtract)  # cross
    dhs = wk.tile([P, F], F32, tag="DHS")
    V._custom_dve(MAD1, out=dhs[:], in0=x3[:], in1=sum2[:], s0=SQ2, s1=0.0)

    # ---- bisector -> (c, s) ----------------------------------------------
    y1 = wk2.tile([P, F], F32, tag="w0")
    G.tensor_tensor(out=y1[:], in0=b1[:], in1=c2p[:], op=OP.mult)
    y2 = wk2.tile([P, F], F32, tag="w1")
    G.tensor_tensor(out=y2[:], in0=b2[:], in1=c1p[:], op=OP.mult)
    ny = wk.tile([P, F], F32, tag="BNY")
    V.tensor_tensor(out=ny[:], in0=y1[:], in1=y2[:], op=OP.add)
    z1 = wk2.tile([P, F], F32, tag="w0")
    G.tensor_tensor(out=z1[:], in0=a1p[:], in1=c2p[:], op=OP.mult)
    z2 = wk2.tile([P, F], F32, tag="w1")
    G.tensor_tensor(out=z2[:], in0=a2p[:], in1=c1p[:], op=OP.mult)
    nx = wk.tile([P, F], F32, tag="BNX")
    V.tensor_tensor(out=nx[:], in0=z1[:], in1=z2[:], op=OP.add)
    n2 = wk2.tile([P, F], F32, tag="w2")
    V._custom_dve(SQSQ, out=n2[:], in0=nx[:], in1=ny[:])
    S.activation(n2[:], n2[:], AF.Ln, bias=EPSN)
    S.activation(n2[:], n2[:], AF.Exp, scale=-0.5)       # rinv
    cc = wk.tile([P, F], F32, tag="BC")
    G.tensor_tensor(out=cc[:], in0=nx[:], in1=n2[:], op=OP.mult)   # c
    ss = wk.tile([P, F], F32, tag="BS")
    G.tensor_tensor(out=ss[:], in0=ny[:], in1=n2[:], op=OP.mult)   # s

    # ---- T chebyshev + SH -------------------------------------------------
    q2 = wk.tile([P, F], F32, tag="TQ")
    S.activation(q2[:], cc[:], AF.Square)
    u_ = wk2.tile([P, F], F32, tag="w0")
    G.tensor_tensor(out=u_[:], in0=ss[:], in1=cc[:], op=OP.mult)
    ta = wk.tile([P, F], F32, tag="TTA")
    V._custom_dve(LINQ1, out=ta[:], in0=cc[:], in1=q2[:],
                  s0=BC0, s1=BC1, imm2=BC2)
    tb = wk2.tile([P, F], F32, tag="w1")
    V._custom_dve(LINQ2, out=tb[:], in0=ss[:], in1=q2[:],
                  s0=CC0, s1=CC1, imm2=CQ2)
    td = wk2.tile([P, F], F32, tag="w2")
    V._custom_dve(LINQ1, out=td[:], in0=u_[:], in1=q2[:],
                  s0=DC0, s1=DC1, imm2=0.0)
    G.tensor_tensor(out=ta[:], in0=ta[:], in1=tb[:], op=OP.add)
    V.tensor_tensor(out=ta[:], in0=ta[:], in1=td[:], op=OP.add)  # T - K0

    ts2t = wk.tile([P, F], F32, tag="TS2")
    G.tensor_tensor(out=ts2t[:], in0=c1p[:], in1=c2p[:], op=OP.add)
    sh = wk.tile([P, F], F32, tag="TSH")
    V._custom_dve(SHF, out=sh[:], in0=ts2t[:], in1=ta[:], s0=0.0075, s1=K0)
    V.reciprocal_approx_fast(out=sh[:], in_=sh[:])       # 1/SH

    # ---- Gaussian + sin poly + Rc ----------------------------------------
    cd = wk2.tile([P, F], F32, tag="w0")
    V._custom_dve(COSD, out=cd[:], in0=cc[:], in1=ss[:], s0=COS275, s1=SIN275)
    S.activation(cd[:], cd[:], AF.Exp, scale=K2, bias=-K2)   # dtheta/30
    V._custom_dve(SINP, out=cd[:], in0=cd[:], s0=math.pi / 3.0,
                  s1=1.0 / 120.0, imm2=-1.0 / 6.0)           # sin(2 dtheta)

    lnt = wk2.tile([P, F], F32, tag="w1")
    S.activation(lnt[:], ts2t[:], AF.Ln, bias=TINY)
    rc = wk.tile([P, F], F32, tag="RC")
    S.activation(rc[:], lnt[:], AF.Exp, scale=7.0, bias=B7)  # Cbarp^7
    S.activation(rc[:], rc[:], AF.Ln, bias=KP7)
    V.scalar_tensor_tensor(out=rc[:], in0=lnt[:], scalar=7.0,
                           in1=rc[:], op0=OP.mult, op1=OP.subtract)
    S.activation(rc[:], rc[:], AF.Exp, scale=0.5, bias=B35)  # Rc/2
    G.tensor_tensor(out=rc[:], in0=rc[:], in1=cd[:], op=OP.mult)  # Rc/2*sin

    # ---- SC, tC, tH, F ----------------------------------------------------
    sc = wk2.tile([P, F], F32, tag="w2")
    S.activation(sc[:], ts2t[:], AF.Copy, scale=0.0225, bias=1.0)
    V.reciprocal_approx_fast(out=sc[:], in_=sc[:])       # 1/SC
    dc = wk2.tile([P, F], F32, tag="w3")
    G.tensor_tensor(out=dc[:], in0=c2p[:], in1=c1p[:], op=OP.subtract)
    G.tensor_tensor(out=dc[:], in0=dc[:], in1=sc[:], op=OP.mult)   # tC
    G.tensor_tensor(out=dhs[:], in0=dhs[:], in1=sh[:], op=OP.mult)  # tHs
    xx = wk2.tile([P, F], F32, tag="w0")
    G.tensor_tensor(out=xx[:], in0=dc[:], in1=dhs[:], op=OP.mult)  # tC*tH
    fa = wk2.tile([P, F], F32, tag="w1")
    V._custom_dve(SQSQ, out=fa[:], in0=dc[:], in1=dhs[:])
    # cross term: -2 * (Rc/2*sin) * (tC*tH)
    V._custom_dve(MAD1, out=xx[:], in0=rc[:], in1=xx[:], s0=-2.0, s1=0.0)
    V.tensor_tensor(out=fa[:], in0=fa[:], in1=tl2[:], op=OP.add)
    G.tensor_tensor(out=fa[:], in0=fa[:], in1=xx[:], op=OP.add)    # F
    S.activation(fa[:], fa[:], AF.Ln, bias=TINY)
    deout = wk2.tile([P, F], F32, tag="w2")
    S.activation(deout[:], fa[:], AF.Exp, scale=0.5,
                 accum_out=acc[:, chunk:chunk + 1])


def _build():
    nc = bacc.Bacc("TRN2", target_bir_lowering=False, debug=False)
    t_out = nc.declare_dram_parameter("outputs", [IPC, C, H, W], F32,
                                      isOutput=False)
    t_lab = nc.declare_dram_parameter("labels", [IPC, C, H, W], F32,
                                      isOutput=False)
    t_part = nc.declare_dram_parameter("partial", [128, NCHUNK], F32,
                                       isOutput=True)
    for i, v in enumerate((TINY, 20.0, KP7, B7, B35, GB, -66.0, -K2, 1.5,
                           1.0, EPSH, EPSN)):
        t = nc.alloc_sbuf_tensor(f"constx{i}", [128, 1], F32)
        nc.gpsimd.memset(t.ap(), v)
        nc.const_aps.aps[(F32, v)] = t.ap()
    nc.all_engine_barrier()
    with tile.TileContext(nc) as tc:
        with tc.tile_pool(name="io", bufs=2) as iop, \
             tc.tile_pool(name="wk", bufs=1) as wk, \
             tc.tile_pool(name="wk2", bufs=2) as wk2, \
             tc.tile_pool(name="accp", bufs=1) as accp:
            acc = accp.tile([128, NCHUNK], F32, tag="acc")
            for img in range(IPC):
                for ci in range(NCH_IMG):
                    chunk = img * NCH_IMG + ci
                    _emit_chunk(nc, iop, wk, wk2, t_out, t_lab,
                                img, ci, acc, chunk)
            nc.sync.dma_start(t_part[:, :], acc[:, :])
    nc.compile()
    return nc


def get_nc():
    if "nc" not in _NC_CACHE:
        _NC_CACHE["nc"] = _build()
    return _NC_CACHE["nc"]


def kernel(outputs: np.ndarray, labels: np.ndarray) -> np.ndarray:
    from concourse.bass_utils import run_bass_kernel_spmd

    outputs = np.ascontiguousarray(outputs, dtype=np.float32)
    labels = np.ascontiguousarray(labels, dtype=np.float32)
    nc = get_nc()
    in_maps = [{"outputs": outputs[i * IPC:(i + 1) * IPC],
                "labels": labels[i * IPC:(i + 1) * IPC]}
               for i in range(NCORE)]
    res = run_bass_kernel_spmd(nc, in_maps, core_ids=list(range(NCORE)))
    total = 0.0
    for r in res.results:
        total += r["partial"].astype(np.float64).sum()
    return np.float32(total / (B * H * W))


if __name__ == "__main__":
    rng = np.random.default_rng(0)
    o = rng.uniform(0, 1, (B, C, H, W)).astype(np.float32)
    l = rng.uniform(0, 1, (B, C, H, W)).astype(np.float32)
    print(kernel(o, l))


# revision 13
# speedup vs baseline: 3.8994x; 1.0532x over previous
"""Trainium2 Bass kernel for nn_ColorLoss: mean CIEDE2000 over RGB images.

Sharding: pure data parallel over batch — 16 images, 8 cores, 2 images/core.
Each core computes per-partition partial sums of deltaE; host reduces.

v2 — fused-custom-DVE restructuring (validated ~1.4e-3 rel in numpy proto):
- ZERO trig activations: hbar never computed. T's four cosines become a
  Chebyshev polynomial in (c, s) = normalized hue-bisector vector; the
  275-degree Gaussian uses (hbar-275)^2 ~ 2(1-cos(h-275))(180/pi)^2; and
  sin(2*dtheta) is a degree-5 odd polynomial. ACT engine therefore stays
  on the natural_log_exp table the whole kernel (no table thrash).
- Signed dH without Sign/select: dH = cross * sqrt(2) * rsqrt(prodC+dot)
  (exact identity: cross^2 = prodC^2 - dot^2).
- sRGB gamma + Lab f() piecewise junctions dropped (dark-pixel-only error,
  ~1.5e-3 on the mean vs 2e-2 tolerance).
- All sqrt/rsqrt/pow via exp(k*ln(x)); reciprocal via custom-DVE
  reciprocal_approx_fast.
- Elementwise chains packed into custom fused DVE ops (<=8 ALU stages, 1
  instruction each); gpsimd used ONLY for plain tensor_tensor (its
  tensor_scalar is ~15us/op on HW - never emitted).
"""
import sys

sys.path.insert(0, '/opt/trn_rl_repo')

import math
import re

import numpy as np

import concourse.bacc as bacc
import concourse.mybir as mybir
import concourse.tile as tile

AF = mybir.ActivationFunctionType
OP = mybir.AluOpType
F32 = mybir.dt.float32

B, C, H, W = 16, 3, 512, 512
NCORE = 8
IPC = B // NCORE            # images per core
PLANE = H * W
PF = PLANE // 128           # 2048 free elems per partition per plane
FCH = 1024                  # free-dim chunk size
NCH_IMG = PF // FCH
NCHUNK = IPC * NCH_IMG

# ---- constants ------------------------------------------------------------
M = [[0.412453, 0.357580, 0.180423],
     [0.212671, 0.715160, 0.072169],
     [0.019334, 0.119193, 0.950227]]
WHITE = [0.95047, 1.0, 1.08883]
EPS = 0.008856
KP7 = 25.0 ** 7
B7 = 7.0 * math.log(0.5)
B35 = 3.5 * math.log(0.5)
TINY = 1e-30
EPSH = 1e-4               # prodC+dot regularizer (antipodal hues)
EPSN = 1e-2               # bisector n2 regularizer
GB = 0.055 / 1.055          # gamma ln bias
K2 = 2.0 * (180.0 / math.pi) ** 2 / 625.0    # Gaussian constant

_c30 = math.cos(math.radians(30))
_c6, _s6 = math.cos(math.radians(6)), math.sin(math.radians(6))
_c63, _s63 = math.cos(math.radians(63)), math.sin(math.radians(63))
# T = K0 + c*(BC0*q+BC1) + BC2*q + CQ2*q^2 + s*(CC0*q+CC1) + u*(DC0*q+DC1)
BC0 = 1.28 * _c6
BC1 = -0.17 * _c30 - 0.96 * _c6
BC2 = 0.48 + 1.6 * _c63
CQ2 = -1.6 * _c63
CC0 = -1.28 * _s6
CC1 = -0.085 + 0.32 * _s6
DC0 = -1.6 * _s63
DC1 = 0.8 * _s63
K0 = 1.0 - 0.24 - 0.2 * _c63
COS275 = math.cos(math.radians(275))
SIN275 = math.sin(math.radians(275))
SQ2 = math.sqrt(2.0)

_NC_CACHE = {}

# ---- pin every activation to the natural_log_exp_and_others table ---------
# The table-load pass otherwise assigns exp->exp_and_others and
# ln->natural_log, inserting a 1.3-1.5us ACT_TABLE_LOAD at every Ln<->Exp
# transition (~106 loads, 163us/core). All activation funcs this kernel
# uses (exp, ln, square, copy) live together in natural_log_exp_and_others,
# so strip them from every other set (keeping dict size/order, hence
# act_func_set_id indices, intact) and the pass has a single legal choice.
_PINNED_SET = "natural_log_exp_and_others"
_PIN_FUNCS = {AF.Exp, AF.Ln, AF.Square, AF.Copy, AF.Identity, AF.Sign,
              AF.Abs}


def _pin_tables(orig):
    def patched(arch):
        tabs = orig(arch)
        out = {}
        for name, funcs in tabs.items():
            if name == _PINNED_SET:
                out[name] = funcs
            else:
                out[name] = {f for f in funcs if f not in _PIN_FUNCS}
        return out
    return patched


import concourse.hw_specs as _hw
import concourse.bacc as _bacc_mod
import concourse.bass_interp as _interp_mod
if not getattr(_hw.get_activation_tables, "_cl_pinned", False):
    _p = _pin_tables(_hw.get_activation_tables)
    _p._cl_pinned = True
    _hw.get_activation_tables = _p
    _bacc_mod.get_activation_tables = _p
    _interp_mod.get_activation_tables = _p

# ---- custom fused DVE ops -------------------------------------------------
from concourse.dve_ops import OPS, CUSTOM_DVE_SPECS, DveOp
from concourse.dve_spec import (Spec, Src0, Src1, C0, C1, C2, One, maxx,
                                relu, sq)


def _mkop(name, body, ref):
    from concourse import dve_ops as _do
    op = DveOp(name, Spec(body=body, reference=ref), subdim=False, uops_sha={})
    OPS.append(op)
    CUSTOM_DVE_SPECS[name] = op.spec
    _do._SUB_OPCODE_FOR_NAME[name] = _do._CUSTOM_DVE_ROW_BASE + len(OPS) - 1
    for ver in ("v3", "v4"):
        try:
            op.compile(ver)
        except ValueError as e:
            m = re.search(r"\(%s: ([0-9a-f]+) " % ver, str(e))
            if m:
                op.uops_sha[ver] = m.group(1)
                op.compile(ver)
        except Exception:
            pass
    return op


SQSQ = _mkop("CL_SQSQ", sq(Src0) + sq(Src1),
             lambda in0, in1, s0, s1, imm2: in0 * in0 + in1 * in1)
MAD1 = _mkop("CL_MAD1", Src0 * Src1 * C0 + C1,
             lambda in0, in1, s0, s1, imm2: in0 * in1 * s0 + s1)
FMAX = _mkop("CL_FMAX", maxx(Src0 * C0 + Src1, C1),
             lambda in0, in1, s0, s1, imm2: np.maximum(in0 * s0 + in1, s1))
SUBMUL = _mkop("CL_SUBMUL", (Src0 - Src1) * C0,
               lambda in0, in1, s0, s1, imm2: (in0 - in1) * s0)
LINQ1 = _mkop("CL_LINQ1", Src0 * (C0 * Src1 + C1) + C2 * Src1,
              lambda in0, in1, s0, s1, imm2: in0 * (s0 * in1 + s1) + imm2 * in1)
LINQ2 = _mkop("CL_LINQ2", Src0 * (C0 * Src1 + C1) + C2 * sq(Src1),
              lambda in0, in1, s0, s1, imm2:
              in0 * (s0 * in1 + s1) + imm2 * in1 * in1)
COSD = _mkop("CL_COSD", C0 * Src0 + C1 * Src1,
             lambda in0, in1, s0, s1, imm2: s0 * in0 + s1 * in1)
_y = C0 * Src0
_y2 = _y * _y
SINP = _mkop("CL_SINP", _y * ((C1 * _y2 + C2) * _y2 + One),
             lambda in0, in1, s0, s1, imm2:
             (s0 * in0) * ((s1 * (s0 * in0) ** 2 + imm2) * (s0 * in0) ** 2
                           + 1.0))
SHF = _mkop("CL_SHF", (Src1 + C1) * C0 * Src0 + One,
            lambda in0, in1, s0, s1, imm2: (in1 + s1) * s0 * in0 + 1.0)
ADDREL = _mkop("CL_ADDREL", relu(Src0 + Src1),
               lambda in0, in1, s0, s1, imm2: np.maximum(in0 + in1, 0.0))


def _emit_lab(nc, wk, wkb, wk2, planes, sFY, sA, sB):
    """RGB planes -> (fy, a, b) tiles. L never materialized (fy carried)."""
    P, F = 128, FCH
    V, S = nc.vector, nc.scalar
    lins = []
    for ci, cp in enumerate(planes):
        lin = wk.tile([P, F], F32, tag=f"lin{ci}")
        # lin = ((c+0.055)/1.055)^2.4  (junction dropped)
        S.activation(lin[:], cp[:], AF.Ln, scale=1.0 / 1.055, bias=GB)
        S.activation(lin[:], lin[:], AF.Exp, scale=2.4)
        lins.append(lin)
    lr, lg, lb = lins
    fs = []
    for k, tag in ((0, "fx"), (1, sFY), (2, "fz")):
        m0, m1, m2 = M[k]
        Sc = m0 / WHITE[k]
        t2 = wk2.tile([P, F], F32, tag="t2")
        # t2 = max(lr + lg*m1/m0 + lb*m2/m0, EPS/Sc)
        V.scalar_tensor_tensor(out=t2[:], in0=lg[:], scalar=m1 / m0,
                               in1=lr[:], op0=OP.mult, op1=OP.add)
        V._custom_dve(FMAX, out=t2[:], in0=lb[:], in1=t2[:],
                      s0=m2 / m0, s1=EPS / Sc)
        f = wk.tile([P, F], F32, tag=tag)
        # f = cbrt(Sc * t2)   (f junction dropped)
        S.activation(f[:], t2[:], AF.Ln, scale=Sc)
        S.activation(f[:], f[:], AF.Exp, scale=1.0 / 3.0)
        fs.append(f)
    fx, fy, fz = fs
    at = wkb.tile([P, F], F32, tag=sA)
    V._custom_dve(SUBMUL, out=at[:], in0=fx[:], in1=fy[:], s0=500.0)
    bt = wkb.tile([P, F], F32, tag=sB)
    V._custom_dve(SUBMUL, out=bt[:], in0=fy[:], in1=fz[:], s0=200.0)
    return fy, at, bt


def _emit_chunk(nc, iop, wk, wkb, psp, wk2, t_out, t_lab, img, ci, acc,
                chunk):
    P, F = 128, FCH
    sl = slice(ci * FCH, (ci + 1) * FCH)

    def load(t_dram, ch, tag):
        view = t_dram[img, ch].rearrange("(p n) w -> p (n w)", p=128)
        tl = iop.tile([P, F], F32, tag=tag)
        nc.sync.dma_start(tl[:], view[:, sl])
        return tl

    lab_planes = [load(t_lab, ch, f"in_l{ch}") for ch in range(3)]
    out_planes = [load(t_out, ch, f"in_o{ch}") for ch in range(3)]

    V, G, S = nc.vector, nc.gpsimd, nc.scalar

    fy1, a1, b1 = _emit_lab(nc, wk, wkb, wk2, lab_planes, "fy1", "sA1",
                            "sB1")
    fy2, a2, b2 = _emit_lab(nc, wk, wkb, wk2, out_planes, "fy2", "sA2",
                            "sB2")

    # ---- L path ----------------------------------------------------------
    s_ = wk2.tile([P, F], F32, tag="w0")
    G.tensor_tensor(out=s_[:], in0=fy1[:], in1=fy2[:], op=OP.add)
    d_ = wk2.tile([P, F], F32, tag="w1")
    G.tensor_tensor(out=d_[:], in0=fy2[:], in1=fy1[:], op=OP.subtract)
    q_ = wk2.tile([P, F], F32, tag="w2")
    S.activation(q_[:], s_[:], AF.Square, scale=58.0, bias=-66.0)
    r_ = wk2.tile([P, F], F32, tag="w0")
    S.activation(r_[:], q_[:], AF.Ln, bias=20.0)
    S.activation(r_[:], r_[:], AF.Exp, scale=-0.5)     # rsqrt(20+q)
    sl_ = wk2.tile([P, F], F32, tag="w3")
    V._custom_dve(MAD1, out=sl_[:], in0=q_[:], in1=r_[:], s0=0.015, s1=1.0)
    V.reciprocal_approx_fast(out=sl_[:], in_=sl_[:])   # 1/SL
    G.tensor_tensor(out=d_[:], in0=d_[:], in1=sl_[:], op=OP.mult)
    tl2 = psp.tile([P, F], F32, tag="TL2")
    S.activation(tl2[:], d_[:], AF.Square, scale=116.0)   # tL^2

    # ---- G (RMS Cbar), a', C' --------------------------------------------
    s1_ = wk2.tile([P, F], F32, tag="w0")
    V._custom_dve(SQSQ, out=s1_[:], in0=a1[:], in1=b1[:])
    s2_ = wk2.tile([P, F], F32, tag="w1")
    V._custom_dve(SQSQ, out=s2_[:], in0=a2[:], in1=b2[:])
    m_ = wk2.tile([P, F], F32, tag="w2")
    G.tensor_tensor(out=m_[:], in0=s1_[:], in1=s2_[:], op=OP.add)
    lm = wk2.tile([P, F], F32, tag="w0")
    S.activation(lm[:], m_[:], AF.Ln, scale=0.5, bias=TINY)  # ln(Cbar_rms^2)
    cb = wk2.tile([P, F], F32, tag="w1")
    S.activation(cb[:], lm[:], AF.Exp, scale=3.5)            # Cbar^7
    S.activation(cb[:], cb[:], AF.Ln, bias=KP7)
    V.scalar_tensor_tensor(out=cb[:], in0=lm[:], scalar=3.5,
                           in1=cb[:], op0=OP.mult, op1=OP.subtract)
    S.activation(cb[:], cb[:], AF.Exp, scale=0.5)            # sqrt term
    gp = wk2.tile([P, F], F32, tag="w2")
    S.activation(gp[:], cb[:], AF.Copy, scale=-0.5, bias=1.5)  # 1+G
    G.tensor_tensor(out=a1[:], in0=a1[:], in1=gp[:], op=OP.mult)   # a1p
    G.tensor_tensor(out=a2[:], in0=a2[:], in1=gp[:], op=OP.mult)   # a2p
    a1p, a2p = a1, a2

    c1s = wk2.tile([P, F], F32, tag="w0")
    V._custom_dve(SQSQ, out=c1s[:], in0=a1p[:], in1=b1[:])   # C1p^2
    c2s = wk2.tile([P, F], F32, tag="w1")
    V._custom_dve(SQSQ, out=c2s[:], in0=a2p[:], in1=b2[:])   # C2p^2
    c1p = wkb.tile([P, F], F32, tag="C1P")
    S.activation(c1p[:], c1s[:], AF.Ln, bias=TINY)
    S.activation(c1p[:], c1p[:], AF.Exp, scale=0.5)          # C1p
    c2p = wkb.tile([P, F], F32, tag="C2P")
    S.activation(c2p[:], c2s[:], AF.Ln, bias=TINY)
    S.activation(c2p[:], c2p[:], AF.Exp, scale=0.5)          # C2p
    prodC = wk2.tile([P, F], F32, tag="w2")
    G.tensor_tensor(out=prodC[:], in0=c1p[:], in1=c2p[:], op=OP.mult)

    # ---- signed dH: cross * sqrt2 * rsqrt(prodC + dot) -------------------
    x1 = wk2.tile([P, F], F32, tag="w0")
    G.tensor_tensor(out=x1[:], in0=a1p[:], in1=a2p[:], op=OP.mult)
    x2 = wk2.tile([P, F], F32, tag="w1")
    G.tensor_tensor(out=x2[:], in0=b1[:], in1=b2[:], op=OP.mult)
    G.tensor_tensor(out=x1[:], in0=x1[:], in1=x2[:], op=OP.add)  # dot
    sum2 = wk2.tile([P, F], F32, tag="w3")
    V._custom_dve(ADDREL, out=sum2[:], in0=prodC[:], in1=x1[:])
    S.activation(sum2[:], sum2[:], AF.Ln, bias=EPSH)
    S.activation(sum2[:], sum2[:], AF.Exp, scale=-0.5)   # rsqrt(prodC+dot)
    x3 = wk2.tile([P, F], F32, tag="w0")
    G.tensor_tensor(out=x3[:], in0=b2[:], in1=a1p[:], op=OP.mult)
    x4 = wk2.tile([P, F], F32, tag="w1")
    G.tensor_tensor(out=x4[:], in0=a2p[:], in1=b1[:], op=OP.mult)
    V.tensor_tensor(out=x3[:], in0=x3[:], in1=x4[:], op=OP.subtract)  # cross
    dhs = wk.tile([P, F], F32, tag="DHS")
    V._custom_dve(MAD1, out=dhs[:], in0=x3[:], in1=sum2[:], s0=SQ2, s1=0.0)

    # ---- bisector -> (c, s) ----------------------------------------------
    y1 = wk2.tile([P, F], F32, tag="w0")
    G.tensor_tensor(out=y1[:], in0=b1[:], in1=c2p[:], op=OP.mult)
    y2 = wk2.tile([P, F], F32, tag="w1")
    G.tensor_tensor(out=y2[:], in0=b2[:], in1=c1p[:], op=OP.mult)
    ny = wk.tile([P, F], F32, tag="BNY")
    V.tensor_tensor(out=ny[:], in0=y1[:], in1=y2[:], op=OP.add)
    z1 = wk2.tile([P, F], F32, tag="w0")
    G.tensor_tensor(out=z1[:], in0=a1p[:], in1=c2p[:], op=OP.mult)
    z2 = wk2.tile([P, F], F32, tag="w1")
    G.tensor_tensor(out=z2[:], in0=a2p[:], in1=c1p[:], op=OP.mult)
    nx = wk.tile([P, F], F32, tag="BNX")
    V.tensor_tensor(out=nx[:], in0=z1[:], in1=z2[:], op=OP.add)
    n2 = wk2.tile([P, F], F32, tag="w2")
    V._custom_dve(SQSQ, out=n2[:], in0=nx[:], in1=ny[:])
    S.activation(n2[:], n2[:], AF.Ln, bias=EPSN)
    S.activation(n2[:], n2[:], AF.Exp, scale=-0.5)       # rinv
    cc = wk.tile([P, F], F32, tag="BC")
    G.tensor_tensor(out=cc[:], in0=nx[:], in1=n2[:], op=OP.mult)   # c
    ss = wk.tile([P, F], F32, tag="BS")
    G.tensor_tensor(out=ss[:], in0=ny[:], in1=n2[:], op=OP.mult)   # s

    # ---- T chebyshev + SH -------------------------------------------------
    q2 = psp.tile([P, F], F32, tag="TQ")
    S.activation(q2[:], cc[:], AF.Square)
    u_ = wk2.tile([P, F], F32, tag="w0")
    G.tensor_tensor(out=u_[:], in0=ss[:], in1=cc[:], op=OP.mult)
    ta = wk.tile([P, F], F32, tag="TTA")
    V._custom_dve(LINQ1, out=ta[:], in0=cc[:], in1=q2[:],
                  s0=BC0, s1=BC1, imm2=BC2)
    tb = wk2.tile([P, F], F32, tag="w1")
    V._custom_dve(LINQ2, out=tb[:], in0=ss[:], in1=q2[:],
                  s0=CC0, s1=CC1, imm2=CQ2)
    td = wk2.tile([P, F], F32, tag="w2")
    V._custom_dve(LINQ1, out=td[:], in0=u_[:], in1=q2[:],
                  s0=DC0, s1=DC1, imm2=0.0)
    G.tensor_tensor(out=ta[:], in0=ta[:], in1=tb[:], op=OP.add)
    V.tensor_tensor(out=ta[:], in0=ta[:], in1=td[:], op=OP.add)  # T - K0

    ts2t = wk.tile([P, F], F32, tag="TS2")
    G.tensor_tensor(out=ts2t[:], in0=c1p[:], in1=c2p[:], op=OP.add)
    sh = psp.tile([P, F], F32, tag="TSH")
    V._custom_dve(SHF, out=sh[:], in0=ts2t[:], in1=ta[:], s0=0.0075, s1=K0)
    V.reciprocal_approx_fast(out=sh[:], in_=sh[:])       # 1/SH

    # ---- Gaussian + sin poly + Rc ----------------------------------------
    cd = wk2.tile([P, F], F32, tag="w0")
    V._custom_dve(COSD, out=cd[:], in0=cc[:], in1=ss[:], s0=COS275, s1=SIN275)
    S.activation(cd[:], cd[:], AF.Exp, scale=K2, bias=-K2)   # dtheta/30
    V._custom_dve(SINP, out=cd[:], in0=cd[:], s0=math.pi / 3.0,
                  s1=1.0 / 120.0, imm2=-1.0 / 6.0)           # sin(2 dtheta)

    lnt = wk2.tile([P, F], F32, tag="w1")
    S.activation(lnt[:], ts2t[:], AF.Ln, bias=TINY)
    rc = psp.tile([P, F], F32, tag="RC")
    S.activation(rc[:], lnt[:], AF.Exp, scale=7.0, bias=B7)  # Cbarp^7
    S.activation(rc[:], rc[:], AF.Ln, bias=KP7)
    V.scalar_tensor_tensor(out=rc[:], in0=lnt[:], scalar=7.0,
                           in1=rc[:], op0=OP.mult, op1=OP.subtract)
    S.activation(rc[:], rc[:], AF.Exp, scale=0.5, bias=B35)  # Rc/2
    V.tensor_tensor(out=rc[:], in0=rc[:], in1=cd[:], op=OP.mult)  # Rc/2*sin

    # ---- SC, tC, tH, F ----------------------------------------------------
    sc = wk2.tile([P, F], F32, tag="w2")
    S.activation(sc[:], ts2t[:], AF.Copy, scale=0.0225, bias=1.0)
    V.reciprocal_approx_fast(out=sc[:], in_=sc[:])       # 1/SC
    dc = wk2.tile([P, F], F32, tag="w3")
    G.tensor_tensor(out=dc[:], in0=c2p[:], in1=c1p[:], op=OP.subtract)
    G.tensor_tensor(out=dc[:], in0=dc[:], in1=sc[:], op=OP.mult)   # tC
    V.tensor_tensor(out=dhs[:], in0=dhs[:], in1=sh[:], op=OP.mult)  # tHs
    xx = wk2.tile([P, F], F32, tag="w0")
    G.tensor_tensor(out=xx[:], in0=dc[:], in1=dhs[:], op=OP.mult)  # tC*tH
    fa = wk2.tile([P, F], F32, tag="w1")
    V._custom_dve(SQSQ, out=fa[:], in0=dc[:], in1=dhs[:])
    # cross term: -2 * (Rc/2*sin) * (tC*tH)
    V._custom_dve(MAD1, out=xx[:], in0=rc[:], in1=xx[:], s0=-2.0, s1=0.0)
    V.tensor_tensor(out=fa[:], in0=fa[:], in1=tl2[:], op=OP.add)
    G.tensor_tensor(out=fa[:], in0=fa[:], in1=xx[:], op=OP.add)    # F
    S.activation(fa[:], fa[:], AF.Ln, bias=TINY)
    deout = wk2.tile([P, F], F32, tag="w2")
    S.activation(deout[:], fa[:], AF.Exp, scale=0.5,
                 accum_out=acc[:, chunk:chunk + 1])


def _build():
    nc = bacc.Bacc("TRN2", target_bir_lowering=False, debug=False)
    t_out = nc.declare_dram_parameter("outputs", [IPC, C, H, W], F32,
                                      isOutput=False)
    t_lab = nc.declare_dram_parameter("labels", [IPC, C, H, W], F32,
                                      isOutput=False)
    t_part = nc.declare_dram_parameter("partial", [128, NCHUNK], F32,
                                       isOutput=True)
    for i, v in enumerate((TINY, 20.0, KP7, B7, B35, GB, -66.0, -K2, 1.5,
                           1.0, EPSH, EPSN)):
        t = nc.alloc_sbuf_tensor(f"constx{i}", [128, 1], F32)
        nc.gpsimd.memset(t.ap(), v)
        nc.const_aps.aps[(F32, v)] = t.ap()
    nc.all_engine_barrier()
    with tile.TileContext(nc) as tc:
        with tc.tile_pool(name="io", bufs=2) as iop, \
             tc.tile_pool(name="wk", bufs=1) as wk, \
             tc.tile_pool(name="wkb", bufs=2) as wkb, \
             tc.tile_pool(name="ps", bufs=1, space="PSUM") as psp, \
             tc.tile_pool(name="wk2", bufs=2) as wk2, \
             tc.tile_pool(name="accp", bufs=1) as accp:
            acc = accp.tile([128, NCHUNK], F32, tag="acc")
            for img in range(IPC):
                for ci in range(NCH_IMG):
                    chunk = img * NCH_IMG + ci
                    _emit_chunk(nc, iop, wk, wkb, psp, wk2, t_out, t_lab,
                                img, ci, acc, chunk)
            nc.sync.dma_start(t_part[:, :], acc[:, :])
    nc.compile()
    return nc


def get_nc():
    if "nc" not in _NC_CACHE:
        _NC_CACHE["nc"] = _build()
    return _NC_CACHE["nc"]


def kernel(outputs: np.ndarray, labels: np.ndarray) -> np.ndarray:
    from concourse.bass_utils import run_bass_kernel_spmd

    outputs = np.ascontiguousarray(outputs, dtype=np.float32)
    labels = np.ascontiguousarray(labels, dtype=np.float32)
    nc = get_nc()
    in_maps = [{"outputs": outputs[i * IPC:(i + 1) * IPC],
                "labels": labels[i * IPC:(i + 1) * IPC]}
               for i in range(NCORE)]
    res = run_bass_kernel_spmd(nc, in_maps, core_ids=list(range(NCORE)))
    total = 0.0
    for r in res.results:
        total += r["partial"].astype(np.float64).sum()
    return np.float32(total / (B * H * W))


if __name__ == "__main__":
    rng = np.random.default_rng(0)
    o = rng.uniform(0, 1, (B, C, H, W)).astype(np.float32)
    l = rng.uniform(0, 1, (B, C, H, W)).astype(np.float32)
    print(kernel(o, l))
